# revision 22
# baseline (speedup 1.0000x reference)
"""Trainium2 Bass kernel for nn_NetworkAwareClassicalExpert (B=256,T=363,C=180).

Data-parallel over 8 NeuronCores: 32 samples/core. Per-core program processes
samples feature-major (channels-on-partitions), transposing to token-major for
layernorm/softmax row ops.

v2 (fast) design notes:
  - bf16 for all per-token matmuls (merge/qkv/scores/AV/out/ffn/pool);
    f32r for proj/conv/corr (N>=256 streams at 1 cycle/row either way, and
    LDWEIGHTS overlaps the previous matmul's stream, so dtype only matters
    for accuracy + small-N ops).
  - all PE transposes in f32r (1.5 c/r, single LDW pass); transpose outputs
    stay fp32 in PSUM so vector/scalar psum reads are plain f32.
  - every rsqrt is exp(-0.5*ln(v+eps)) on the scalar engine: ln/exp live in
    the same activation table as softmax's exp, so the only table switches
    left are gelu<->ln/exp.
  - samples emitted in blocks of 4 with stage-major ordering inside the
    block, so the scalar queue sees [gelu x4][ln/exp x4][gelu x12]... =
    ~6 table loads per block instead of ~8 per sample.
  - conv bias folded into the GroupNorm affine (mean shifts by the bias,
    per-channel variance is unchanged), saving the ones-row bias matmuls.
  - attention: scores computed transposed [s,t], exp via ACT -> bf16; AV as
    N=33 bf16 matmuls (32ns each) with an appended ones-column giving the
    softmax normalizer Z for free.
  - FC fingerprint: block means of the 180x180 correlation collapse to a
    12x12 Gram of per-network invstd-weighted channel sums + rank-1 mean
    correction.
"""

import sys
import os

sys.path.insert(0, "/opt/trn_rl_repo")

import numpy as np

import concourse.bass as bass
import concourse.mybir as mybir
import concourse.tile as tile
import bass_rust
from concourse.vector_clock import ScopedClock
from concourse.masks import make_identity

F32 = mybir.dt.float32
F32R = mybir.dt.float32r
BF16 = mybir.dt.bfloat16
AF = mybir.ActivationFunctionType
ALU = mybir.AluOpType

B, T, C = 256, 363, 180
CD = 96
DM = 128
DILS = (1, 4, 16)
NCORES = 8
S = int(os.environ.get("KB_NSAMP", str(B // NCORES)))
TCH = 121                # t-chunk (3 chunks of 121)
PAD = 48
EPS = 1e-5
ISQ = float(1.0 / np.sqrt(32.0))
BLK = 4

DEBUG = bool(int(os.environ.get("KBDBG", "0")))
DBG_SAMPLES = int(os.environ.get("KBDBG_S", "2"))


def _patch_tile_drain():
    """This walrus rejects >1 sem wait on the final Tile drain: split them."""

    def _drain_and_barrier(self, tick_clock, wait_clock):
        drain_inst = self.nc.sync.drain()
        wait_clock.add_sem_waits(
            drain_inst.ins, ScopedClock({None: tick_clock.global_clock})
        )
        si = drain_inst.ins.sync_info
        if si is not None and si.on_wait is not None and len(si.on_wait) > 1:
            waits = list(si.on_wait)
            ups = list(si.on_update) if si.on_update else []
            drain_inst.ins.sync_info = bass_rust.SyncInfo(
                on_wait=waits[:1], on_update=ups
            )
            for w in waits[1:]:
                nop = self.nc.sync.nop()
                nop.ins.sync_info = bass_rust.SyncInfo(on_wait=[w], on_update=[])
        self.nc.all_engine_barrier()
        popped = self.nc._tile_sem_poison_stack.pop()
        assert popped is self._sem_poison
        if not int(os.environ.get("KB_NOSEMCLEAR", "0")):
            self.nc.clear_and_free_semaphores(list(self.sems.allocated().values()))
        self.nc.all_engine_barrier()

    tile.TileContext._drain_and_barrier = _drain_and_barrier


_patch_tile_drain()


def nn_cur_bb(nc):
    bbw = nc.cur_bb
    return bbw.bb if hasattr(bbw, "bb") else bbw


def _split_sync_waits(nc, max_waits=1):
    """walrus rejects instructions with >1 sem wait; hoist excess onto
    same-engine NOPs inserted immediately before."""
    for f in nc.m.functions:
        for bb in f.blocks:
            insts = list(bb.instructions)
            out = []
            changed = False
            for inst in insts:
                si = getattr(inst, "sync_info", None)
                if si is not None and si.on_wait and len(si.on_wait) > max_waits:
                    waits = list(si.on_wait)
                    ups = list(si.on_update) if si.on_update else []
                    extra = waits[max_waits:]
                    for i in range(0, len(extra), max_waits):
                        nop = nc.engines[inst.engine].nop(nofuse=True)
                        cur = nn_cur_bb(nc)
                        lst = list(cur.instructions)
                        assert lst and lst[-1].name == nop.ins.name
                        cur.instructions = lst[:-1]
                        nop.ins.sync_info = bass_rust.SyncInfo(
                            on_wait=extra[i:i + max_waits], on_update=[])
                        out.append(nop.ins)
                    inst.sync_info = bass_rust.SyncInfo(
                        on_wait=waits[:max_waits], on_update=ups)
                    changed = True
                out.append(inst)
            if changed:
                bb.instructions = out


INPUT_SPECS = [
    ("x", (S, T, C)),
    ("w_proj", (12, 15, 8)), ("b_proj", (12, 8)),
    ("dw_w", (3, 96, 7)), ("dw_b", (3, 96)),
    ("pw_w", (3, 96, 96)), ("pw_b", (3, 96)),
    ("gn_g", (3, 96)), ("gn_b", (3, 96)),
    ("merge_w", (288, 128)), ("merge_b", (128,)),
    ("merge_ln_g", (128,)), ("merge_ln_b", (128,)),
    ("qkv_w", (384, 128)), ("qkv_b", (384,)),
    ("out_w", (128, 128)), ("out_b", (128,)),
    ("ln1_g", (128,)), ("ln1_b", (128,)),
    ("ff1_w", (256, 128)), ("ff1_b", (256,)),
    ("ff2_w", (128, 256)), ("ff2_b", (128,)),
    ("ln2_g", (128,)), ("ln2_b", (128,)),
    ("pool_w", (128, 1)), ("pool_b", (1,)),
    ("fc_w", (78, 64)), ("fc_b", (64,)),
    ("fus1_w", (192, 128)), ("fus1_b", (128,)),
    ("fus_ln_g", (128,)), ("fus_ln_b", (128,)),
    ("fus2_w", (128, 64)), ("fus2_b", (64,)),
]


def build_program():
    nc = bass.Bass("TRN2", target_bir_lowering=False, debug=False,
                   num_devices=NCORES)
    D = {}
    for name, shape in INPUT_SPECS:
        D[name] = nc.dram_tensor(name, list(shape), F32, kind="ExternalInput").ap()
    out_dram = nc.dram_tensor("out", [S, 64], F32, kind="ExternalOutput").ap()
    dbg_shapes = {}

    with tile.TileContext(nc) as tc:
        with nc.allow_low_precision(reason="deliberate bf16/f32r pipeline"):
            _build(nc, tc, D, out_dram, dbg_shapes)
    if not int(os.environ.get("KB_NOSPLIT", "0")):
        _split_sync_waits(nc)
    return nc, dbg_shapes


def _build(nc, tc, D, out_dram, dbg_shapes):
    pools = []

    def mkpool(name, bufs, space="SBUF"):
        p = tc.tile_pool(name=name, bufs=bufs, space=space)
        pools.append(p)
        return p.__enter__()

    W = mkpool("weights", 1)        # persistent tiles, one tag each
    sb = mkpool("sb", 6)            # per-sample P1 state (block depth 4 + lag)
    p2 = mkpool("p2", 2)            # per-sample P2 transients
    tiny = mkpool("tiny", 8)        # small per-sample stats
    big = mkpool("big", 1, "PSUM")  # 3-bank psum class (conv/qkv/scores/ffn)
    pp = mkpool("pp", 5, "PSUM")    # 1-bank psum misc

    dma = nc.sync

    _pa_n = [0]

    def pa(shape, dtype=F32):
        _pa_n[0] += 1
        return pp.tile(list(shape), dtype, tag="a", name=f"pa{_pa_n[0]}")

    _bg_n = [0]

    def pbig(shape, dtype=F32):
        _bg_n[0] += 1
        return big.tile(list(shape), dtype, tag="big", name=f"pb{_bg_n[0]}")

    def dbg(name, ap, shape):
        if not DEBUG:
            return
        t = nc.dram_tensor(f"dbg_{name}", list(shape), F32,
                           kind="ExternalOutput").ap()
        dbg_shapes[name] = tuple(shape)
        if ap.dtype != F32:
            tmp = sb.tile(list(shape), F32, tag=f"dbgt_{name}")
            nc.vector.tensor_copy(tmp, ap)
            dma.dma_start(out=t, in_=tmp)
        else:
            dma.dma_start(out=t, in_=ap)

    # ================= weight preload =================
    ident = W.tile([128, 128], F32, tag="ident")
    make_identity(nc, ident)
    identb = W.tile([128, 128], BF16, tag="identb")
    nc.vector.tensor_copy(identb, ident)
    ones32 = W.tile([1, 32], BF16, tag="ones32")
    nc.vector.memset(ones32, 1.0)

    wprojf = W.tile([90, 2, 96], F32, tag="wprojf")
    nc.vector.memset(wprojf, 0.0)
    for n in range(12):
        g, j = divmod(n, 6)
        dma.dma_start(out=wprojf[j * 15:(j + 1) * 15, g, n * 8:(n + 1) * 8],
                      in_=D["w_proj"][n])
    wproj = W.tile([90, 2, 96], BF16, tag="wproj")
    nc.vector.tensor_copy(wproj, wprojf)
    bproj = W.tile([96, 1], F32, tag="bproj")
    dma.dma_start(out=bproj, in_=D["b_proj"].rearrange("a b -> (a b)").unsqueeze(1))

    pwT, dwk = [], []
    for k in range(3):
        t_ = W.tile([96, 96], F32, tag=f"pwT{k}")
        dma.dma_start(out=t_, in_=D["pw_w"][k].transpose([1, 0]))
        pwT.append(t_)
        t2 = W.tile([96, 7], F32, tag=f"dw{k}")
        dma.dma_start(out=t2, in_=D["dw_w"][k])
        dwk.append(t2)
    wconv = []
    for k in range(3):
        t_ = W.tile([96, 7, 96], F32R, tag=f"wconv{k}")
        for j in range(7):
            nc.vector.tensor_scalar_mul(t_[:, j, :], pwT[k], dwk[k][:, j:j + 1])
        wconv.append(t_)
    dwb = W.tile([96, 3], F32, tag="dwb")
    dma.dma_start(out=dwb, in_=D["dw_b"].transpose([1, 0]))
    pwb = W.tile([96, 3], F32, tag="pwb")
    dma.dma_start(out=pwb, in_=D["pw_b"].transpose([1, 0]))
    cb_ps = pa([96, 3])
    for k in range(3):
        nc.tensor.matmul(cb_ps[:, k:k + 1], pwT[k], dwb[:, k:k + 1],
                         start=True, stop=True, skip_group_check=True)
    cb = W.tile([96, 3], F32, tag="cb")
    nc.vector.tensor_add(cb, cb_ps, pwb)

    gng = W.tile([96, 3], F32, tag="gng")
    dma.dma_start(out=gng, in_=D["gn_g"].transpose([1, 0]))
    gnb = W.tile([96, 3], F32, tag="gnb")
    dma.dma_start(out=gnb, in_=D["gn_b"].transpose([1, 0]))

    # wgrp[c, g] = 1/12 iff 0 <= c - 12g <= 11 ; wbc[g, c] = 1 iff same
    wgrp = W.tile([96, 8], F32, tag="wgrp")
    nc.vector.memset(wgrp, 1.0 / 12.0)
    nc.gpsimd.affine_select(out=wgrp, in_=wgrp, compare_op=ALU.is_ge,
                            fill=0.0, base=0, pattern=[[-12, 8]],
                            channel_multiplier=1)
    nc.gpsimd.affine_select(out=wgrp, in_=wgrp, compare_op=ALU.is_ge,
                            fill=0.0, base=11, pattern=[[12, 8]],
                            channel_multiplier=-1)
    wbc = W.tile([8, 96], F32, tag="wbc")
    nc.vector.memset(wbc, 1.0)
    nc.gpsimd.affine_select(out=wbc, in_=wbc, compare_op=ALU.is_ge,
                            fill=0.0, base=0, pattern=[[1, 96]],
                            channel_multiplier=-12)
    nc.gpsimd.affine_select(out=wbc, in_=wbc, compare_op=ALU.is_ge,
                            fill=0.0, base=11, pattern=[[-1, 96]],
                            channel_multiplier=12)

    wst = W.tile([128, 128], F32, tag="wst")  # staging for bf16 casts

    mw = []
    for g in range(3):
        t_ = W.tile([96, 128], BF16, tag=f"mw{g}")
        dma.dma_start(out=wst[0:96, :], in_=D["merge_w"][g * 96:(g + 1) * 96, :])
        nc.vector.tensor_copy(t_, wst[0:96, :])
        mw.append(t_)
    mb = W.tile([128, 1], F32, tag="mb")
    dma.dma_start(out=mb, in_=D["merge_b"].unsqueeze(1))
    mlng = W.tile([128, 1], F32, tag="mlng")
    dma.dma_start(out=mlng, in_=D["merge_ln_g"].unsqueeze(1))
    mlnb = W.tile([128, 1], F32, tag="mlnb")
    dma.dma_start(out=mlnb, in_=D["merge_ln_b"].unsqueeze(1))

    qkvT = []
    for i in range(3):
        t_ = W.tile([128, 128], BF16, tag=f"qkvT{i}")
        dma.dma_start(out=wst,
                      in_=D["qkv_w"][i * 128:(i + 1) * 128, :].transpose([1, 0]))
        nc.vector.tensor_copy(t_, wst)
        qkvT.append(t_)
    qb3 = W.tile([128, 3], F32, tag="qb3")
    dma.dma_start(out=qb3, in_=D["qkv_b"].rearrange("(a b) -> b a", a=3))
    qb_s = W.tile([128, 1], F32, tag="qb_s")
    nc.vector.tensor_scalar_mul(qb_s, qb3[:, 0:1], ISQ)

    owT_s = []
    dma.dma_start(out=wst, in_=D["out_w"].transpose([1, 0]))
    for p in range(2):
        t_ = W.tile([97, 128], BF16, tag=f"owTs{p}")
        nc.vector.memset(t_, 0.0)
        nc.vector.tensor_copy(t_[0:32, :], wst[p * 64:p * 64 + 32, :])
        nc.vector.tensor_copy(t_[64:96, :], wst[p * 64 + 32:p * 64 + 64, :])
        owT_s.append(t_)
    ob = W.tile([128, 1], F32, tag="ob")
    dma.dma_start(out=ob, in_=D["out_b"].unsqueeze(1))

    ln1g = W.tile([128, 1], F32, tag="ln1g")
    dma.dma_start(out=ln1g, in_=D["ln1_g"].unsqueeze(1))
    ln1b = W.tile([128, 1], F32, tag="ln1b")
    dma.dma_start(out=ln1b, in_=D["ln1_b"].unsqueeze(1))
    ln2g = W.tile([128, 1], F32, tag="ln2g")
    dma.dma_start(out=ln2g, in_=D["ln2_g"].unsqueeze(1))
    ln2b = W.tile([128, 1], F32, tag="ln2b")
    dma.dma_start(out=ln2b, in_=D["ln2_b"].unsqueeze(1))

    f1T, f2T = [], []
    for i in range(2):
        t_ = W.tile([128, 128], BF16, tag=f"f1T{i}")
        dma.dma_start(out=wst,
                      in_=D["ff1_w"][i * 128:(i + 1) * 128, :].transpose([1, 0]))
        nc.vector.tensor_copy(t_, wst)
        f1T.append(t_)
        t2 = W.tile([128, 128], BF16, tag=f"f2T{i}")
        dma.dma_start(out=wst,
                      in_=D["ff2_w"][:, i * 128:(i + 1) * 128].transpose([1, 0]))
        nc.vector.tensor_copy(t2, wst)
        f2T.append(t2)
    f1b = W.tile([128, 2], F32, tag="f1b")
    dma.dma_start(out=f1b, in_=D["ff1_b"].rearrange("(a b) -> b a", a=2))
    f2b = W.tile([128, 1], F32, tag="f2b")
    dma.dma_start(out=f2b, in_=D["ff2_b"].unsqueeze(1))

    poolw = W.tile([128, 1], BF16, tag="poolw")
    dma.dma_start(out=wst[:, 0:1], in_=D["pool_w"])
    nc.vector.tensor_copy(poolw, wst[:, 0:1])
    poolb = W.tile([1, 1], F32, tag="poolb")
    dma.dma_start(out=poolb, in_=D["pool_b"].unsqueeze(1))

    # cmask[p, g, n] = 1 iff 0 <= p - 15*(n - 6g) <= 14
    cmask = W.tile([90, 2, 12], F32, tag="cmask")
    nc.vector.memset(cmask, 1.0)
    nc.gpsimd.affine_select(out=cmask, in_=cmask, compare_op=ALU.is_ge,
                            fill=0.0, base=0, pattern=[[90, 2], [-15, 12]],
                            channel_multiplier=1)
    nc.gpsimd.affine_select(out=cmask, in_=cmask, compare_op=ALU.is_ge,
                            fill=0.0, base=14, pattern=[[-90, 2], [15, 12]],
                            channel_multiplier=-1)
    kcorr = float(1.0 / (15 * 15 * (T - 1)))
    fcw = W.tile([78, 64], F32, tag="fcw")
    dma.dma_start(out=fcw, in_=D["fc_w"])
    fcwk = W.tile([78, 64], F32, tag="fcwk")
    nc.vector.tensor_scalar_mul(fcwk, fcw, kcorr)
    fcb = W.tile([64, 1], F32, tag="fcb")
    dma.dma_start(out=fcb, in_=D["fc_b"].unsqueeze(1))

    fu1T = W.tile([128, 2, 128], F32, tag="fu1T")
    nc.vector.memset(fu1T[:, 1, :], 0.0)
    dma.dma_start(out=fu1T[:, 0, :], in_=D["fus1_w"][0:128, :])
    dma.dma_start(out=fu1T[0:64, 1, :], in_=D["fus1_w"][128:192, :])
    fu1b = W.tile([128, 1], F32, tag="fu1b")
    dma.dma_start(out=fu1b, in_=D["fus1_b"].unsqueeze(1))
    flg = W.tile([128, 1], F32, tag="flg")
    dma.dma_start(out=flg, in_=D["fus_ln_g"].unsqueeze(1))
    flb = W.tile([128, 1], F32, tag="flb")
    dma.dma_start(out=flb, in_=D["fus_ln_b"].unsqueeze(1))
    fu2T = W.tile([128, 64], F32, tag="fu2T")
    dma.dma_start(out=fu2T, in_=D["fus2_w"])
    fu2b = W.tile([64, 1], F32, tag="fu2b")
    dma.dma_start(out=fu2b, in_=D["fus2_b"].unsqueeze(1))

    epst = W.tile([128, 1], F32, tag="epst")
    nc.vector.memset(epst, EPS)

    bm_all = W.tile([12, S, 12], F32, tag="bm_all")
    fus_t = W.tile([128, S], F32, tag="fus_t")
    fus_f = W.tile([64, S], F32, tag="fus_f")
    P2STOP = int(os.environ.get("KB_P2STOP", "9"))
    if P2STOP != 9:
        nc.vector.memset(bm_all, 0.1)
        nc.vector.memset(fus_t, 0.1)

    # ================= per-sample stages =================
    st = [dict() for _ in range(S)]

    def stage_load(s):
        """DMA x, transpose to feature-major, x stats, proj matmul."""
        d = st[s]
        xtm = sb.tile([TCH, 3, C], F32, tag="xtm")
        dma.dma_start(out=xtm, in_=D["x"][s].rearrange("(c p) f -> p c f", p=TCH))
        xtmb = sb.tile([TCH, 3, C], BF16, tag="xtmb")
        nc.vector.tensor_copy(xtmb, xtm)
        xfm = sb.tile([90, 2, 364], BF16, tag="xfm")
        nc.vector.memset(xfm[:, :, 363:364], 0.0)
        for g in range(2):
            xps = pa([90, 3, 128], BF16)
            for c in range(3):
                nc.tensor.transpose(xps[:, c, 0:TCH],
                                    xtmb[:, c, g * 90:(g + 1) * 90],
                                    identb[0:TCH, 0:TCH])
            nc.vector.tensor_copy(
                xfm[:, g, 0:363].rearrange("a (c t) -> a c t", c=3),
                xps[:, :, 0:TCH])
        xst = tiny.tile([90, 2, 3, 6], F32, tag="xst")
        for g in range(2):
            for c in range(3):
                nc.vector.bn_stats(
                    xst[:, g, c],
                    xfm[:, g, c * TCH:(c + 1) * TCH])
        xmv = tiny.tile([90, 2, 2], F32, tag="xmv")
        for g in range(2):
            nc.vector.bn_aggr(xmv[:, g], xst[:, g])
        d["xfm"], d["xmv"] = xfm, xmv
        hps = pa([96, 512])
        for g in range(2):
            nc.tensor.matmul(hps[:, 0:364], wproj[:, g, :], xfm[:, g],
                             start=(g == 0), stop=(g == 1))
        d["hps"] = hps

    def stage_hpad(s):
        """[G] gelu(proj + bias) into padded conv input row."""
        d = st[s]
        hpad = sb.tile([96, PAD + T + PAD + 5], F32R, tag="hpad")
        nc.vector.memset(hpad.bitcast(F32)[:, 0:PAD], 0.0)
        nc.vector.memset(hpad.bitcast(F32)[:, PAD + T:], 0.0)
        nc.scalar.activation(hpad[:, PAD:PAD + T], d["hps"][:, 0:T], AF.Gelu,
                             bias=bproj, scale=1.0)
        d["hpad"] = hpad
        if DEBUG and s < DBG_SAMPLES:
            dbg(f"h{s}", hpad.bitcast(F32)[:, PAD:PAD + T], (96, T))

    def stage_conv(s):
        """dw-folded-into-pw conv (no bias), cast to bf16, per-channel stats,
        group aggregation. Conv bias folds into the GN affine."""
        d = st[s]
        yps = pbig([96, 3, 512])
        for k in range(3):
            dd = DILS[k]
            for j in range(7):
                off = PAD + (j - 3) * dd
                nc.tensor.matmul(yps[:, k, 0:364], wconv[k][:, j, :],
                                 d["hpad"][:, off:off + 364],
                                 start=(j == 0), stop=(j == 6))
        ysb = sb.tile([96, 3, 364], BF16, tag="ysb")
        nc.vector.tensor_copy(ysb[:, :, 0:T], yps[:, :, 0:T])
        yst = tiny.tile([96, 3, 6], F32, tag="yst")
        for k in range(3):
            nc.vector.bn_stats(yst[:, k], ysb[:, k, 0:T])
        ymv = tiny.tile([96, 3, 2], F32, tag="ymv")
        for k in range(3):
            nc.vector.bn_aggr(ymv[:, k], yst[:, k])
        # st6: [mean + conv_bias, E[(y+b)^2]] per channel
        st6 = tiny.tile([96, 6], F32, tag="st6")
        nc.vector.tensor_tensor(st6[:, 0:3], ymv[:, :, 0], cb, op=ALU.add)
        nc.vector.tensor_tensor(st6[:, 3:6], st6[:, 0:3], st6[:, 0:3],
                                op=ALU.mult)
        nc.vector.tensor_tensor(st6[:, 3:6], st6[:, 3:6], ymv[:, :, 1],
                                op=ALU.add)
        gst_ps = pa([8, 6])
        nc.tensor.matmul(gst_ps, wgrp, st6, start=True, stop=True)
        gst = tiny.tile([8, 6], F32, tag="gst")
        nc.vector.tensor_copy(gst, gst_ps)
        gvar = tiny.tile([8, 3], F32, tag="gvar")
        nc.vector.tensor_tensor(gvar, gst[:, 0:3], gst[:, 0:3], op=ALU.mult)
        nc.vector.tensor_tensor(gvar, gst[:, 3:6], gvar, op=ALU.subtract)
        d["ysb"], d["gst"], d["gvar"] = ysb, gst, gvar

    def stage_gn_rsqrt(s):
        """[LnExp] group inv-std = exp(-0.5 ln(var+eps)); also corr channel
        inv-std from x stats."""
        d = st[s]
        gln = tiny.tile([8, 3], F32, tag="gln")
        nc.scalar.activation(gln, d["gvar"], AF.Ln, bias=epst[0:8, :], scale=1.0)
        nc.scalar.activation(d["gst"][:, 3:6], gln, AF.Exp, scale=-0.5)
        # corr: cinv = 1/max(sqrt(var*T/(T-1)), 1e-8)
        cvar = tiny.tile([90, 2], F32, tag="cvar")
        nc.vector.tensor_scalar_max(cvar, d["xmv"][:, :, 1], 1e-16)
        cln = tiny.tile([90, 2], F32, tag="cln")
        nc.scalar.activation(cln, cvar, AF.Ln, bias=0.0,
                             scale=float(T) / (T - 1))
        cinv = tiny.tile([90, 2], F32, tag="cinv")
        nc.scalar.activation(cinv, cln, AF.Exp, scale=-0.5)
        d["cinv"] = cinv

    def stage_gn_affine(s):
        """broadcast group stats to channels; per-channel scale/bias with the
        conv bias folded in: out = gelu(y*scl + bia),
        scl = g/sd_g, bia = gnb - (M_g - cb)*scl."""
        d = st[s]
        bc_ps = pa([96, 6])
        nc.tensor.matmul(bc_ps, wbc, d["gst"], start=True, stop=True)
        bc = tiny.tile([96, 6], F32, tag="bc")
        nc.vector.tensor_copy(bc, bc_ps)
        scl = tiny.tile([96, 3], F32, tag="scl")
        nc.vector.tensor_tensor(scl, gng, bc[:, 3:6], op=ALU.mult)
        bia = tiny.tile([96, 3], F32, tag="bia")
        nc.vector.tensor_tensor(bia, bc[:, 0:3], cb, op=ALU.subtract)
        nc.vector.tensor_tensor(bia, bia, scl, op=ALU.mult)
        nc.vector.tensor_tensor(bia, gnb, bia, op=ALU.subtract)
        d["scl"], d["bia"] = scl, bia

    def stage_cat(s):
        """[G] cat = gelu(GN(y)) in bf16."""
        d = st[s]
        cat = sb.tile([96, 3, 364], BF16, tag="cat")
        nc.vector.memset(cat[:, :, 363:364], 0.0)
        for k in range(3):
            nc.scalar.activation(cat[:, k, 0:T], d["ysb"][:, k, 0:T], AF.Gelu,
                                 bias=d["bia"][:, k:k + 1],
                                 scale=d["scl"][:, k:k + 1])
        d["cat"] = cat
        if DEBUG and s < DBG_SAMPLES:
            dbg(f"cat{s}", cat[:, :, 0:T], (96, 3 * T))

    def stage_merge(s):
        """merge matmul, bias, transpose to token-major, LN stats."""
        d = st[s]
        ups = pa([128, 512])
        for g in range(3):
            nc.tensor.matmul(ups[:, 0:364], mw[g], d["cat"][:, g],
                             start=(g == 0), stop=(g == 2))
        ufm = sb.tile([128, 364], BF16, tag="ufm")
        nc.scalar.activation(ufm[:, 0:T], ups[:, 0:T], AF.Identity,
                             bias=mb, scale=1.0)
        tpm = pa([TCH, 3, 128], BF16)
        for c in range(3):
            nc.tensor.transpose(tpm[:, c, :],
                                ufm[:, c * TCH:(c + 1) * TCH], identb)
        mst = tiny.tile([TCH, 3, 6], F32, tag="mst")
        for c in range(3):
            nc.vector.bn_stats(mst[:, c], tpm[:, c])
        mmv = tiny.tile([TCH, 3, 2], F32, tag="mmv")
        for c in range(3):
            nc.vector.bn_aggr(mmv[:, c], mst[:, c])
        d["tpm"], d["mmv"] = tpm, mmv

    def stage_mln_rsqrt(s):
        """[LnExp] token inv-std for merge LN."""
        d = st[s]
        mln_t = tiny.tile([TCH, 3], F32, tag="mln_t")
        nc.scalar.activation(mln_t, d["mmv"][:, :, 1], AF.Ln,
                             bias=epst[0:TCH, :], scale=1.0)
        mrs = tiny.tile([TCH, 3], F32, tag="mrs")
        nc.scalar.activation(mrs, mln_t, AF.Exp, scale=-0.5)
        d["mrs"] = mrs

    def stage_h0(s):
        """normalize, transpose back, [G] h0 = gelu(hat*g+b) in bf16."""
        d = st[s]
        hatm = sb.tile([TCH, 3, 128], BF16, tag="hatm")
        for c in range(3):
            nc.vector.tensor_scalar(hatm[:, c], d["tpm"][:, c],
                                    d["mmv"][:, c, 0:1], d["mrs"][:, c:c + 1],
                                    op0=ALU.subtract, op1=ALU.mult)
        tp2 = pa([128, 3, 128], BF16)
        for c in range(3):
            nc.tensor.transpose(tp2[:, c, 0:TCH], hatm[:, c, :],
                                identb[0:TCH, 0:TCH])
        h0 = sb.tile([128, 364], BF16, tag="h0")
        nc.vector.memset(h0[:, 363:364], 0.0)
        nc.scalar.activation(
            h0[:, 0:363].rearrange("a (c t) -> a c t", c=3), tp2[:, :, 0:TCH],
            AF.Gelu, bias=mlnb, scale=mlng)
        d["h0"] = h0
        if DEBUG and s < DBG_SAMPLES:
            dbg(f"h0_{s}", h0[:, 0:T], (128, T))

    def ln_tm(s, src_fm, pfx, keep_hat=False):
        """transpose fm->tm, stats, [LnExp] inv-std, normalize (f32r hat)."""
        d = st[s]
        tp = pa([TCH, 3, 128], BF16)
        for c in range(3):
            nc.tensor.transpose(tp[:, c, :],
                                src_fm[:, c * TCH:(c + 1) * TCH], identb)
        lst = tiny.tile([TCH, 3, 6], F32, tag=f"{pfx}st")
        for c in range(3):
            nc.vector.bn_stats(lst[:, c], tp[:, c])
        lmv = tiny.tile([TCH, 3, 2], F32, tag=f"{pfx}mv")
        for c in range(3):
            nc.vector.bn_aggr(lmv[:, c], lst[:, c])
        lln = tiny.tile([TCH, 3], F32, tag=f"{pfx}ln")
        nc.scalar.activation(lln, lmv[:, :, 1], AF.Ln,
                             bias=epst[0:TCH, :], scale=1.0)
        lrs = tiny.tile([TCH, 3], F32, tag=f"{pfx}rs")
        nc.scalar.activation(lrs, lln, AF.Exp, scale=-0.5)
        pool_ = p2 if keep_hat else sb
        hat = pool_.tile([TCH, 3, 128], BF16, tag=f"{pfx}hat")
        for c in range(3):
            nc.vector.tensor_scalar(hat[:, c], tp[:, c],
                                    lmv[:, c, 0:1], lrs[:, c:c + 1],
                                    op0=ALU.subtract, op1=ALU.mult)
        tpb = pa([128, 3, 128], BF16)
        for c in range(3):
            nc.tensor.transpose(tpb[:, c, 0:TCH], hat[:, c, :],
                                identb[0:TCH, 0:TCH])
        return hat, tpb

    def stage_p2(s):
        """[LnExp table throughout] qkv -> attention -> out-proj -> ln1 ->
        ffn -> ln2 -> attentive pooling -> correlation gram."""
        d = st[s]
        h0 = d["h0"]
        if P2STOP == 0:
            return
        # ---- qkv ----
        qkvps = pbig([128, 3, 512])
        for i in range(3):
            nc.tensor.matmul(qkvps[:, i, 0:364], qkvT[i], h0,
                             start=True, stop=True)
        qfm = p2.tile([64, 2, 364], BF16, tag="qfm")
        kfm = p2.tile([64, 2, 364], BF16, tag="kfm")
        nc.vector.memset(qfm[:, :, 363:364], 0.0)
        nc.vector.memset(kfm[:, :, 363:364], 0.0)
        for i in range(2):
            nc.scalar.activation(qfm[:, i, 0:T],
                                 qkvps[i * 64:(i + 1) * 64, 0, 0:T],
                                 AF.Identity, bias=qb_s[i * 64:(i + 1) * 64, :],
                                 scale=ISQ)
            nc.vector.tensor_scalar(kfm[:, i, 0:T],
                                    qkvps[i * 64:(i + 1) * 64, 1, 0:T],
                                    1.0, qb3[i * 64:(i + 1) * 64, 1:2],
                                    op0=ALU.mult, op1=ALU.add)
        vfm = p2.tile([128, 364], BF16, tag="vfm")
        nc.vector.tensor_scalar(vfm[:, 0:T], qkvps[:, 2, 0:T], 1.0, qb3[:, 2:3],
                                op0=ALU.mult, op1=ALU.add)
        vtp = pa([TCH, 3, 128], BF16)
        for c in range(3):
            nc.tensor.transpose(vtp[:, c, :], vfm[:, c * TCH:(c + 1) * TCH],
                                identb)
        vtm = p2.tile([TCH, 3, 4, 33], BF16, tag="vtm")
        nc.vector.tensor_copy(vtm[:, :, :, 0:32],
                              vtp.rearrange("p c (h d) -> p c h d", h=4))
        nc.vector.memset(vtm[:, :, :, 32:33], 1.0)

        if P2STOP == 1:
            return
        # ---- scores (transposed) + exp (bf16) ----
        expt = p2.tile([TCH, 3, 2, 2, 364], BF16, tag="expt")
        for cs in range(3):
            for b_ in range(2):
                scps = pbig([TCH, 2, 512])
                for i in range(2):
                    nc.tensor.matmul(
                        scps[:, i, 0:364],
                        kfm[b_ * 32:(b_ + 1) * 32, i,
                            cs * TCH:(cs + 1) * TCH],
                        qfm[b_ * 32:(b_ + 1) * 32, i, :],
                        start=True, stop=True)
                nc.scalar.activation(
                    expt[:, cs, b_], scps[:, :, 0:364], AF.Exp)

        if P2STOP == 2:
            return
        # ---- AV: V stationary (33-col LDW), exp moving N=364; out is
        # feature-major per-head [33,364] with Z on row 32; heads packed two
        # per 1-bank psum tile at partition offsets 0/64. ----
        avps = [pa([97, 512]), pa([97, 512])]
        for h in range(4):
            tile_ = avps[h // 2]
            po = (h % 2) * 64
            for cs in range(3):
                nc.tensor.matmul(
                    tile_[po:po + 33, 0:364],
                    vtm[:, cs, h, :],
                    expt[:, cs, h % 2, h // 2, :],
                    start=(cs == 0), stop=(cs == 2))
        # 1/Z rows -> one sbuf row-tile (free-dim stacked), PE-broadcast to
        # 32 partitions per head, then normalize O per head.
        zrow = p2.tile([1, 4, 364], F32, tag="zrow")
        for h in range(4):
            nc.scalar.activation(
                zrow[:, h, :],
                avps[h // 2][(h % 2) * 64 + 32:(h % 2) * 64 + 33, 0:364],
                AF.Identity)
        rzrowf = p2.tile([1, 4, 364], F32, tag="rzrowf")
        nc.vector.reciprocal(rzrowf, zrow)
        rzrow = p2.tile([1, 4, 364], BF16, tag="rzrow")
        nc.vector.tensor_copy(rzrow, rzrowf)
        rzb = pbig([97, 2, 512])
        for h in range(4):
            nc.tensor.matmul(rzb[(h % 2) * 64:(h % 2) * 64 + 32, h // 2, 0:364],
                             ones32, rzrow[:, h, :], start=True, stop=True)
        rzbs = p2.tile([97, 2, 364], BF16, tag="rzbs")
        nc.vector.tensor_copy(rzbs[0:32], rzb[0:32, :, 0:364])
        nc.vector.tensor_copy(rzbs[64:96], rzb[64:96, :, 0:364])
        oat = p2.tile([97, 2, 364], BF16, tag="oat")
        nc.vector.memset(oat, 0.0)
        for h in range(4):
            po = (h % 2) * 64
            nc.vector.tensor_tensor(oat[po:po + 32, h // 2, :],
                                    avps[h // 2][po:po + 32, 0:364],
                                    rzbs[po:po + 32, h // 2, :], op=ALU.mult)

        if P2STOP == 3:
            return
        # ---- out proj (4 per-head accumulating matmuls) + residual ----
        rps = pa([128, 512])
        for p_ in range(2):
            nc.tensor.matmul(rps[:, 0:364], owT_s[p_], oat[:, p_, :],
                             start=(p_ == 0), stop=(p_ == 1))
        if P2STOP == 31:
            return
        rfm = p2.tile([128, 364], BF16, tag="rfm")
        nc.vector.scalar_tensor_tensor(rfm[:, 0:T], rps[:, 0:T], ob,
                                       h0[:, 0:T], op0=ALU.add, op1=ALU.add)
        if P2STOP == 32:
            return
        if DEBUG and s < DBG_SAMPLES:
            dbg(f"rfm{s}", rfm[:, 0:T], (128, T))
        hat1, tpb1 = ln_tm(s, rfm, "l1", keep_hat=False)
        if P2STOP == 33:
            return
        h1 = p2.tile([128, 364], BF16, tag="h1")
        nc.vector.memset(h1[:, 363:364], 0.0)
        nc.scalar.activation(
            h1[:, 0:363].rearrange("a (c t) -> a c t", c=3), tpb1[:, :, 0:TCH],
            AF.Identity, bias=ln1b, scale=ln1g)

        if P2STOP == 4:
            return
        # ---- ffn ----
        f1ps = pbig([128, 2, 512])
        for i in range(2):
            nc.tensor.matmul(f1ps[:, i, 0:364], f1T[i], h1,
                             start=True, stop=True)
        g1 = p2.tile([128, 2, 364], BF16, tag="g1")
        nc.vector.memset(g1[:, :, 363:364], 0.0)
        for i in range(2):
            nc.scalar.activation(g1[:, i, 0:T], f1ps[:, i, 0:T], AF.Relu,
                                 bias=f1b[:, i:i + 1], scale=1.0)
        f2ps = pa([128, 512])
        for i in range(2):
            nc.tensor.matmul(f2ps[:, 0:364], f2T[i], g1[:, i],
                             start=(i == 0), stop=(i == 1))
        ffo = p2.tile([128, 364], BF16, tag="ffo")
        nc.vector.scalar_tensor_tensor(ffo[:, 0:T], f2ps[:, 0:T], f2b,
                                       h1[:, 0:T], op0=ALU.add, op1=ALU.add)
        hat2, tpb2 = ln_tm(s, ffo, "l2", keep_hat=True)
        h2 = p2.tile([128, 364], BF16, tag="h2")
        nc.vector.memset(h2[:, 363:364], 0.0)
        nc.scalar.activation(
            h2[:, 0:363].rearrange("a (c t) -> a c t", c=3), tpb2[:, :, 0:TCH],
            AF.Identity, bias=ln2b, scale=ln2g)
        if DEBUG and s < DBG_SAMPLES:
            dbg(f"h2_{s}", h2[:, 0:T], (128, T))

        if P2STOP == 5:
            return
        # ---- attentive pooling ----
        plps = pa([1, 512])
        nc.tensor.matmul(plps[:, 0:364], poolw, h2, start=True, stop=True)
        pw_sb = p2.tile([1, T], F32, tag="pw_sb")
        zp = tiny.tile([1, 1], F32, tag="zp")
        nc.scalar.activation(pw_sb, plps[:, 0:T], AF.Exp,
                             bias=poolb, scale=1.0, accum_out=zp)
        rzp = tiny.tile([1, 1], F32, tag="rzp")
        nc.vector.reciprocal(rzp, zp)
        wn = p2.tile([1, T], BF16, tag="wn")
        nc.vector.tensor_scalar_mul(wn, pw_sb, rzp)
        wtp = pa([TCH, 3, 2], BF16)
        for c in range(3):
            nc.tensor.transpose(wtp[:, c, 0:1],
                                wn[:, c * TCH:(c + 1) * TCH],
                                identb[0:1, 0:1])
        wcol = tiny.tile([TCH, 3, 1], BF16, tag="wcol")
        nc.vector.tensor_copy(wcol, wtp[:, :, 0:1])
        tps = pa([128, 1])
        for c in range(3):
            nc.tensor.matmul(tps, hat2[:, c, :], wcol[:, c, :],
                             start=(c == 0), stop=(c == 2))
        nc.vector.tensor_scalar(fus_t[:, s:s + 1], tps, ln2g, ln2b,
                                op0=ALU.mult, op1=ALU.add)

        if P2STOP == 6:
            return
        # ---- correlation fingerprint ----
        wcorr = p2.tile([90, 2, 12], BF16, tag="wcorr")
        for g in range(2):
            nc.vector.tensor_scalar_mul(wcorr[:, g], cmask[:, g],
                                        d["cinv"][:, g:g + 1])
        swps = pa([12, 512])
        for g in range(2):
            nc.tensor.matmul(swps[:, 0:364], wcorr[:, g], d["xfm"][:, g],
                             start=(g == 0), stop=(g == 1))
        swsb = p2.tile([12, T], BF16, tag="swsb")
        rsum = tiny.tile([12, 1], F32, tag="rsum")
        nc.vector.tensor_scalar(swsb, swps[:, 0:T], 1.0, 0.0, op0=ALU.mult,
                                op1=ALU.add, accum_out=rsum)
        swtp = pa([TCH, 3, 12], BF16)
        for c in range(3):
            nc.tensor.transpose(swtp[:, c, :], swsb[:, c * TCH:(c + 1) * TCH],
                                identb[0:12, 0:12])
        swtm = tiny.tile([TCH, 3, 12], BF16, tag="swtm")
        nc.vector.tensor_copy(swtm, swtp)
        rsT_ps = pa([1, 12])
        nc.tensor.transpose(rsT_ps, rsum, ident[0:12, 0:12])
        rsT = tiny.tile([1, 12], F32, tag="rsT")
        nc.vector.tensor_copy(rsT, rsT_ps)
        rsTn = tiny.tile([1, 12], F32, tag="rsTn")
        nc.vector.tensor_scalar_mul(rsTn, rsT, -1.0 / T)
        gps = pa([12, 12])
        for c in range(3):
            nc.tensor.matmul(gps, swtm[:, c, :], swtm[:, c, :],
                             start=(c == 0), stop=False)
        nc.tensor.matmul(gps, rsTn, rsT, start=False, stop=True)
        nc.vector.tensor_copy(bm_all[:, s, :], gps)

    # ================= block-scheduled emission =================
    for b0 in range(0, S, BLK):
        blk = range(b0, min(b0 + BLK, S))
        for s in blk:
            stage_load(s)
        for s in blk:
            stage_hpad(s)       # [G]
        for s in blk:
            stage_conv(s)
        for s in blk:
            stage_gn_rsqrt(s)   # [LnExp]
        for s in blk:
            stage_gn_affine(s)
        for s in blk:
            stage_cat(s)        # [G]
        for s in blk:
            stage_merge(s)
        for s in blk:
            stage_mln_rsqrt(s)  # [LnExp]
        for s in blk:
            stage_h0(s)         # [G]
        for s in blk:
            stage_p2(s)         # [LnExp]

    # ================= batched tail =================
    bm_dram = nc.dram_tensor("bm_scratch", [12, S, 12], F32).ap()
    dma.dma_start(out=bm_dram, in_=bm_all)
    fcv = W.tile([78, S], F32, tag="fcv")
    row_off = 0
    for i in range(12):
        n = 12 - i
        dma.dma_start(
            out=fcv[row_off:row_off + n, :],
            in_=bm_dram[i, :, i:12].transpose([1, 0]))
        row_off += n
    fcps = pa([64, S])
    nc.tensor.matmul(fcps, fcwk, fcv, start=True, stop=True)
    nc.scalar.activation(fus_f, fcps, AF.Gelu, bias=fcb, scale=1.0)

    fu_ps = pa([128, S])
    nc.tensor.matmul(fu_ps, fu1T[:, 0, :], fus_t, start=True, stop=False)
    nc.tensor.matmul(fu_ps, fu1T[0:64, 1, :], fus_f, start=False, stop=True)
    zfm = W.tile([128, S], F32, tag="zfm")
    nc.vector.tensor_scalar(zfm, fu_ps, 1.0, fu1b, op0=ALU.mult, op1=ALU.add)
    ztp = pa([S, 128])
    nc.tensor.transpose(ztp, zfm, ident)
    ztm = W.tile([S, 128], F32, tag="ztm")
    nc.vector.tensor_copy(ztm, ztp)
    zst = W.tile([S, 6], F32, tag="zst")
    nc.vector.bn_stats(zst, ztm)
    zmv = W.tile([S, 2], F32, tag="zmv")
    nc.vector.bn_aggr(zmv, zst)
    zln = W.tile([S, 1], F32, tag="zln")
    nc.scalar.activation(zln, zmv[:, 1:2], AF.Ln, bias=epst[0:S, :], scale=1.0)
    zrs = W.tile([S, 1], F32, tag="zrs")
    nc.scalar.activation(zrs, zln, AF.Exp, scale=-0.5)
    zhat = W.tile([S, 128], F32, tag="zhat")
    nc.vector.tensor_scalar(zhat, ztm, zmv[:, 0:1], zrs,
                            op0=ALU.subtract, op1=ALU.mult)
    zhtp = pa([128, S])
    nc.tensor.transpose(zhtp, zhat, ident[0:S, 0:S])
    zg = W.tile([128, S], F32, tag="zg")
    nc.scalar.activation(zg, zhtp, AF.Gelu, bias=flb, scale=flg)
    out_ps = pa([64, S])
    nc.tensor.matmul(out_ps, fu2T, zg, start=True, stop=True)
    out_sb = W.tile([64, S], F32, tag="out_sb")
    nc.scalar.activation(out_sb, out_ps, AF.Identity, bias=fu2b, scale=1.0)
    outT_ps = pa([S, 64])
    nc.tensor.transpose(outT_ps, out_sb, ident[0:64, 0:64])
    outT = W.tile([S, 64], F32, tag="outT")
    nc.vector.tensor_copy(outT, outT_ps)
    dma.dma_start(out=out_dram, in_=outT)

    for p in reversed(pools):
        p.__exit__(None, None, None)


_PROGRAM = None


def _get_program():
    global _PROGRAM
    if _PROGRAM is None:
        _PROGRAM = build_program()
    return _PROGRAM


def kernel(**inputs):
    from concourse.bass_utils import run_bass_kernel_spmd

    nc, _ = _get_program()
    in_maps = []
    for c in range(NCORES):
        m = {}
        for name, _shape in INPUT_SPECS:
            if name == "x":
                m["x"] = np.ascontiguousarray(
                    np.asarray(inputs["x"][c * S:(c + 1) * S], dtype=np.float32))
            else:
                m[name] = np.ascontiguousarray(
                    np.asarray(inputs[name], dtype=np.float32))
        in_maps.append(m)
    res = run_bass_kernel_spmd(nc, in_maps, list(range(NCORES)))
    global LAST_RESULTS
    LAST_RESULTS = res
    out = np.concatenate([res.results[c]["out"] for c in range(NCORES)], axis=0)
    return out.astype(np.float32)


LAST_RESULTS = None


# revision 23
# speedup vs baseline: 1.1333x; 1.1333x over previous
"""Trainium2 Bass kernel for nn_NetworkAwareClassicalExpert (B=256,T=363,C=180).

Data-parallel over 8 NeuronCores: 32 samples/core. Per-core program processes
samples feature-major (channels-on-partitions), transposing to token-major for
layernorm/softmax row ops.

v2 (fast) design notes:
  - bf16 for all per-token matmuls (merge/qkv/scores/AV/out/ffn/pool);
    f32r for proj/conv/corr (N>=256 streams at 1 cycle/row either way, and
    LDWEIGHTS overlaps the previous matmul's stream, so dtype only matters
    for accuracy + small-N ops).
  - all PE transposes in f32r (1.5 c/r, single LDW pass); transpose outputs
    stay fp32 in PSUM so vector/scalar psum reads are plain f32.
  - every rsqrt is exp(-0.5*ln(v+eps)) on the scalar engine: ln/exp live in
    the same activation table as softmax's exp, so the only table switches
    left are gelu<->ln/exp.
  - samples emitted in blocks of 4 with stage-major ordering inside the
    block, so the scalar queue sees [gelu x4][ln/exp x4][gelu x12]... =
    ~6 table loads per block instead of ~8 per sample.
  - conv bias folded into the GroupNorm affine (mean shifts by the bias,
    per-channel variance is unchanged), saving the ones-row bias matmuls.
  - attention: scores computed transposed [s,t], exp via ACT -> bf16; AV as
    N=33 bf16 matmuls (32ns each) with an appended ones-column giving the
    softmax normalizer Z for free.
  - FC fingerprint: block means of the 180x180 correlation collapse to a
    12x12 Gram of per-network invstd-weighted channel sums + rank-1 mean
    correction.
"""

import sys
import os

sys.path.insert(0, "/opt/trn_rl_repo")

import numpy as np

import concourse.bass as bass
import concourse.mybir as mybir
import concourse.tile as tile
import bass_rust
from concourse.vector_clock import ScopedClock
from concourse.masks import make_identity

F32 = mybir.dt.float32
F32R = mybir.dt.float32r
BF16 = mybir.dt.bfloat16
AF = mybir.ActivationFunctionType
ALU = mybir.AluOpType

B, T, C = 256, 363, 180
CD = 96
DM = 128
DILS = (1, 4, 16)
NCORES = 8
S = int(os.environ.get("KB_NSAMP", str(B // NCORES)))
TCH = 121                # t-chunk (3 chunks of 121)
PAD = 48
EPS = 1e-5
ISQ = float(1.0 / np.sqrt(32.0))
BLK = 4

DEBUG = bool(int(os.environ.get("KBDBG", "0")))
DBG_SAMPLES = int(os.environ.get("KBDBG_S", "2"))


def _patch_tile_drain():
    """This walrus rejects >1 sem wait on the final Tile drain: split them."""

    def _drain_and_barrier(self, tick_clock, wait_clock):
        drain_inst = self.nc.sync.drain()
        wait_clock.add_sem_waits(
            drain_inst.ins, ScopedClock({None: tick_clock.global_clock})
        )
        si = drain_inst.ins.sync_info
        if si is not None and si.on_wait is not None and len(si.on_wait) > 1:
            waits = list(si.on_wait)
            ups = list(si.on_update) if si.on_update else []
            drain_inst.ins.sync_info = bass_rust.SyncInfo(
                on_wait=waits[:1], on_update=ups
            )
            for w in waits[1:]:
                nop = self.nc.sync.nop()
                nop.ins.sync_info = bass_rust.SyncInfo(on_wait=[w], on_update=[])
        self.nc.all_engine_barrier()
        popped = self.nc._tile_sem_poison_stack.pop()
        assert popped is self._sem_poison
        if not int(os.environ.get("KB_NOSEMCLEAR", "0")):
            self.nc.clear_and_free_semaphores(list(self.sems.allocated().values()))
        self.nc.all_engine_barrier()

    tile.TileContext._drain_and_barrier = _drain_and_barrier


_patch_tile_drain()


def nn_cur_bb(nc):
    bbw = nc.cur_bb
    return bbw.bb if hasattr(bbw, "bb") else bbw


def _split_sync_waits(nc, max_waits=1):
    """walrus rejects instructions with >1 sem wait; hoist excess onto
    same-engine NOPs inserted immediately before."""
    for f in nc.m.functions:
        for bb in f.blocks:
            insts = list(bb.instructions)
            out = []
            changed = False
            for inst in insts:
                si = getattr(inst, "sync_info", None)
                if si is not None and si.on_wait and len(si.on_wait) > max_waits:
                    waits = list(si.on_wait)
                    ups = list(si.on_update) if si.on_update else []
                    extra = waits[max_waits:]
                    for i in range(0, len(extra), max_waits):
                        nop = nc.engines[inst.engine].nop(nofuse=True)
                        cur = nn_cur_bb(nc)
                        lst = list(cur.instructions)
                        assert lst and lst[-1].name == nop.ins.name
                        cur.instructions = lst[:-1]
                        nop.ins.sync_info = bass_rust.SyncInfo(
                            on_wait=extra[i:i + max_waits], on_update=[])
                        out.append(nop.ins)
                    inst.sync_info = bass_rust.SyncInfo(
                        on_wait=waits[:max_waits], on_update=ups)
                    changed = True
                out.append(inst)
            if changed:
                bb.instructions = out


INPUT_SPECS = [
    ("x", (S, T, C)),
    ("w_proj", (12, 15, 8)), ("b_proj", (12, 8)),
    ("dw_w", (3, 96, 7)), ("dw_b", (3, 96)),
    ("pw_w", (3, 96, 96)), ("pw_b", (3, 96)),
    ("gn_g", (3, 96)), ("gn_b", (3, 96)),
    ("merge_w", (288, 128)), ("merge_b", (128,)),
    ("merge_ln_g", (128,)), ("merge_ln_b", (128,)),
    ("qkv_w", (384, 128)), ("qkv_b", (384,)),
    ("out_w", (128, 128)), ("out_b", (128,)),
    ("ln1_g", (128,)), ("ln1_b", (128,)),
    ("ff1_w", (256, 128)), ("ff1_b", (256,)),
    ("ff2_w", (128, 256)), ("ff2_b", (128,)),
    ("ln2_g", (128,)), ("ln2_b", (128,)),
    ("pool_w", (128, 1)), ("pool_b", (1,)),
    ("fc_w", (78, 64)), ("fc_b", (64,)),
    ("fus1_w", (192, 128)), ("fus1_b", (128,)),
    ("fus_ln_g", (128,)), ("fus_ln_b", (128,)),
    ("fus2_w", (128, 64)), ("fus2_b", (64,)),
]


def build_program():
    nc = bass.Bass("TRN2", target_bir_lowering=False, debug=False,
                   num_devices=NCORES)
    D = {}
    for name, shape in INPUT_SPECS:
        D[name] = nc.dram_tensor(name, list(shape), F32, kind="ExternalInput").ap()
    out_dram = nc.dram_tensor("out", [S, 64], F32, kind="ExternalOutput").ap()
    dbg_shapes = {}

    with tile.TileContext(nc) as tc:
        with nc.allow_low_precision(reason="deliberate bf16/f32r pipeline"):
            _build(nc, tc, D, out_dram, dbg_shapes)
    if not int(os.environ.get("KB_NOSPLIT", "0")):
        _split_sync_waits(nc)
    return nc, dbg_shapes


def _build(nc, tc, D, out_dram, dbg_shapes):
    pools = []

    def mkpool(name, bufs, space="SBUF"):
        p = tc.tile_pool(name=name, bufs=bufs, space=space)
        pools.append(p)
        return p.__enter__()

    W = mkpool("weights", 1)        # persistent tiles, one tag each
    sb = mkpool("sb", 6)            # per-sample P1 state (block depth 4 + lag)
    p2 = mkpool("p2", 2)            # per-sample P2 transients
    tiny = mkpool("tiny", 8)        # small per-sample stats
    pp = mkpool("pp", 4, "PSUM")    # unified psum pool: 4 x 4KB buffers

    dma = nc.sync

    _pa_n = [0]

    def pa(shape, dtype=F32):
        _pa_n[0] += 1
        return pp.tile(list(shape), dtype, tag="a", name=f"pa{_pa_n[0]}")

    def dbg(name, ap, shape):
        if not DEBUG:
            return
        t = nc.dram_tensor(f"dbg_{name}", list(shape), F32,
                           kind="ExternalOutput").ap()
        dbg_shapes[name] = tuple(shape)
        if ap.dtype != F32:
            tmp = sb.tile(list(shape), F32, tag=f"dbgt_{name}")
            nc.vector.tensor_copy(tmp, ap)
            dma.dma_start(out=t, in_=tmp)
        else:
            dma.dma_start(out=t, in_=ap)

    # ================= weight preload =================
    ident = W.tile([128, 128], F32, tag="ident")
    make_identity(nc, ident)
    identb = W.tile([128, 128], BF16, tag="identb")
    nc.vector.tensor_copy(identb, ident)
    ones32 = W.tile([1, 32], BF16, tag="ones32")
    nc.vector.memset(ones32, 1.0)

    wprojf = W.tile([90, 2, 96], F32, tag="wprojf")
    nc.vector.memset(wprojf, 0.0)
    for n in range(12):
        g, j = divmod(n, 6)
        dma.dma_start(out=wprojf[j * 15:(j + 1) * 15, g, n * 8:(n + 1) * 8],
                      in_=D["w_proj"][n])
    wproj = W.tile([90, 2, 96], BF16, tag="wproj")
    nc.vector.tensor_copy(wproj, wprojf)
    bproj = W.tile([96, 1], F32, tag="bproj")
    dma.dma_start(out=bproj, in_=D["b_proj"].rearrange("a b -> (a b)").unsqueeze(1))

    pwT, dwk = [], []
    for k in range(3):
        t_ = W.tile([96, 96], F32, tag=f"pwT{k}")
        dma.dma_start(out=t_, in_=D["pw_w"][k].transpose([1, 0]))
        pwT.append(t_)
        t2 = W.tile([96, 7], F32, tag=f"dw{k}")
        dma.dma_start(out=t2, in_=D["dw_w"][k])
        dwk.append(t2)
    wconv = []
    for k in range(3):
        t_ = W.tile([96, 7, 96], F32R, tag=f"wconv{k}")
        for j in range(7):
            nc.vector.tensor_scalar_mul(t_[:, j, :], pwT[k], dwk[k][:, j:j + 1])
        wconv.append(t_)
    dwb = W.tile([96, 3], F32, tag="dwb")
    dma.dma_start(out=dwb, in_=D["dw_b"].transpose([1, 0]))
    pwb = W.tile([96, 3], F32, tag="pwb")
    dma.dma_start(out=pwb, in_=D["pw_b"].transpose([1, 0]))
    cb_ps = pa([96, 3])
    for k in range(3):
        nc.tensor.matmul(cb_ps[:, k:k + 1], pwT[k], dwb[:, k:k + 1],
                         start=True, stop=True, skip_group_check=True)
    cb = W.tile([96, 3], F32, tag="cb")
    nc.vector.tensor_add(cb, cb_ps, pwb)

    gng = W.tile([96, 3], F32, tag="gng")
    dma.dma_start(out=gng, in_=D["gn_g"].transpose([1, 0]))
    gnb = W.tile([96, 3], F32, tag="gnb")
    dma.dma_start(out=gnb, in_=D["gn_b"].transpose([1, 0]))

    # wgrp[c, g] = 1/12 iff 0 <= c - 12g <= 11 ; wbc[g, c] = 1 iff same
    wgrp = W.tile([96, 8], F32, tag="wgrp")
    nc.vector.memset(wgrp, 1.0 / 12.0)
    nc.gpsimd.affine_select(out=wgrp, in_=wgrp, compare_op=ALU.is_ge,
                            fill=0.0, base=0, pattern=[[-12, 8]],
                            channel_multiplier=1)
    nc.gpsimd.affine_select(out=wgrp, in_=wgrp, compare_op=ALU.is_ge,
                            fill=0.0, base=11, pattern=[[12, 8]],
                            channel_multiplier=-1)
    wbc = W.tile([8, 96], F32, tag="wbc")
    nc.vector.memset(wbc, 1.0)
    nc.gpsimd.affine_select(out=wbc, in_=wbc, compare_op=ALU.is_ge,
                            fill=0.0, base=0, pattern=[[1, 96]],
                            channel_multiplier=-12)
    nc.gpsimd.affine_select(out=wbc, in_=wbc, compare_op=ALU.is_ge,
                            fill=0.0, base=11, pattern=[[-1, 96]],
                            channel_multiplier=12)

    wst = W.tile([128, 128], F32, tag="wst")  # staging for bf16 casts

    mw = []
    for g in range(3):
        t_ = W.tile([96, 128], BF16, tag=f"mw{g}")
        dma.dma_start(out=wst[0:96, :], in_=D["merge_w"][g * 96:(g + 1) * 96, :])
        nc.vector.tensor_copy(t_, wst[0:96, :])
        mw.append(t_)
    mb = W.tile([128, 1], F32, tag="mb")
    dma.dma_start(out=mb, in_=D["merge_b"].unsqueeze(1))
    mlng = W.tile([128, 1], F32, tag="mlng")
    dma.dma_start(out=mlng, in_=D["merge_ln_g"].unsqueeze(1))
    mlnb = W.tile([128, 1], F32, tag="mlnb")
    dma.dma_start(out=mlnb, in_=D["merge_ln_b"].unsqueeze(1))

    qkvT = []
    for i in range(3):
        t_ = W.tile([128, 128], BF16, tag=f"qkvT{i}")
        dma.dma_start(out=wst,
                      in_=D["qkv_w"][i * 128:(i + 1) * 128, :].transpose([1, 0]))
        nc.vector.tensor_copy(t_, wst)
        qkvT.append(t_)
    qb3 = W.tile([128, 3], F32, tag="qb3")
    dma.dma_start(out=qb3, in_=D["qkv_b"].rearrange("(a b) -> b a", a=3))
    qb_s = W.tile([128, 1], F32, tag="qb_s")
    nc.vector.tensor_scalar_mul(qb_s, qb3[:, 0:1], ISQ)

    owT_s = []
    dma.dma_start(out=wst, in_=D["out_w"].transpose([1, 0]))
    for p in range(2):
        t_ = W.tile([97, 128], BF16, tag=f"owTs{p}")
        nc.vector.memset(t_, 0.0)
        nc.vector.tensor_copy(t_[0:32, :], wst[p * 64:p * 64 + 32, :])
        nc.vector.tensor_copy(t_[64:96, :], wst[p * 64 + 32:p * 64 + 64, :])
        owT_s.append(t_)
    ob = W.tile([128, 1], F32, tag="ob")
    dma.dma_start(out=ob, in_=D["out_b"].unsqueeze(1))

    ln1g = W.tile([128, 1], F32, tag="ln1g")
    dma.dma_start(out=ln1g, in_=D["ln1_g"].unsqueeze(1))
    ln1b = W.tile([128, 1], F32, tag="ln1b")
    dma.dma_start(out=ln1b, in_=D["ln1_b"].unsqueeze(1))
    ln2g = W.tile([128, 1], F32, tag="ln2g")
    dma.dma_start(out=ln2g, in_=D["ln2_g"].unsqueeze(1))
    ln2b = W.tile([128, 1], F32, tag="ln2b")
    dma.dma_start(out=ln2b, in_=D["ln2_b"].unsqueeze(1))

    f1T, f2T = [], []
    for i in range(2):
        t_ = W.tile([128, 128], BF16, tag=f"f1T{i}")
        dma.dma_start(out=wst,
                      in_=D["ff1_w"][i * 128:(i + 1) * 128, :].transpose([1, 0]))
        nc.vector.tensor_copy(t_, wst)
        f1T.append(t_)
        t2 = W.tile([128, 128], BF16, tag=f"f2T{i}")
        dma.dma_start(out=wst,
                      in_=D["ff2_w"][:, i * 128:(i + 1) * 128].transpose([1, 0]))
        nc.vector.tensor_copy(t2, wst)
        f2T.append(t2)
    f1b = W.tile([128, 2], F32, tag="f1b")
    dma.dma_start(out=f1b, in_=D["ff1_b"].rearrange("(a b) -> b a", a=2))
    f2b = W.tile([128, 1], F32, tag="f2b")
    dma.dma_start(out=f2b, in_=D["ff2_b"].unsqueeze(1))

    poolw = W.tile([128, 1], BF16, tag="poolw")
    dma.dma_start(out=wst[:, 0:1], in_=D["pool_w"])
    nc.vector.tensor_copy(poolw, wst[:, 0:1])
    poolb = W.tile([1, 1], F32, tag="poolb")
    dma.dma_start(out=poolb, in_=D["pool_b"].unsqueeze(1))

    # cmask[p, g, n] = 1 iff 0 <= p - 15*(n - 6g) <= 14
    cmask = W.tile([90, 2, 12], F32, tag="cmask")
    nc.vector.memset(cmask, 1.0)
    nc.gpsimd.affine_select(out=cmask, in_=cmask, compare_op=ALU.is_ge,
                            fill=0.0, base=0, pattern=[[90, 2], [-15, 12]],
                            channel_multiplier=1)
    nc.gpsimd.affine_select(out=cmask, in_=cmask, compare_op=ALU.is_ge,
                            fill=0.0, base=14, pattern=[[-90, 2], [15, 12]],
                            channel_multiplier=-1)
    kcorr = float(1.0 / (15 * 15 * (T - 1)))
    fcw = W.tile([78, 64], F32, tag="fcw")
    dma.dma_start(out=fcw, in_=D["fc_w"])
    fcwk = W.tile([78, 64], F32, tag="fcwk")
    nc.vector.tensor_scalar_mul(fcwk, fcw, kcorr)
    fcb = W.tile([64, 1], F32, tag="fcb")
    dma.dma_start(out=fcb, in_=D["fc_b"].unsqueeze(1))

    fu1T = W.tile([128, 2, 128], F32, tag="fu1T")
    nc.vector.memset(fu1T[:, 1, :], 0.0)
    dma.dma_start(out=fu1T[:, 0, :], in_=D["fus1_w"][0:128, :])
    dma.dma_start(out=fu1T[0:64, 1, :], in_=D["fus1_w"][128:192, :])
    fu1b = W.tile([128, 1], F32, tag="fu1b")
    dma.dma_start(out=fu1b, in_=D["fus1_b"].unsqueeze(1))
    flg = W.tile([128, 1], F32, tag="flg")
    dma.dma_start(out=flg, in_=D["fus_ln_g"].unsqueeze(1))
    flb = W.tile([128, 1], F32, tag="flb")
    dma.dma_start(out=flb, in_=D["fus_ln_b"].unsqueeze(1))
    fu2T = W.tile([128, 64], F32, tag="fu2T")
    dma.dma_start(out=fu2T, in_=D["fus2_w"])
    fu2b = W.tile([64, 1], F32, tag="fu2b")
    dma.dma_start(out=fu2b, in_=D["fus2_b"].unsqueeze(1))

    epst = W.tile([128, 1], F32, tag="epst")
    nc.vector.memset(epst, EPS)

    bm_all = W.tile([12, S, 12], F32, tag="bm_all")
    fus_t = W.tile([128, S], F32, tag="fus_t")
    fus_f = W.tile([64, S], F32, tag="fus_f")
    P2STOP = int(os.environ.get("KB_P2STOP", "9"))
    if P2STOP != 9:
        nc.vector.memset(bm_all, 0.1)
        nc.vector.memset(fus_t, 0.1)

    # ================= per-sample stages =================
    st = [dict() for _ in range(S)]

    def stage_load(s):
        """DMA x, transpose to feature-major, x stats, proj matmul."""
        d = st[s]
        xtm = sb.tile([TCH, 3, C], F32, tag="xtm")
        dma.dma_start(out=xtm, in_=D["x"][s].rearrange("(c p) f -> p c f", p=TCH))
        xtmb = sb.tile([TCH, 3, C], BF16, tag="xtmb")
        nc.vector.tensor_copy(xtmb, xtm)
        xfm = sb.tile([90, 2, 364], BF16, tag="xfm")
        nc.vector.memset(xfm[:, :, 363:364], 0.0)
        for g in range(2):
            xps = pa([90, 3, 128], BF16)
            for c in range(3):
                nc.tensor.transpose(xps[:, c, 0:TCH],
                                    xtmb[:, c, g * 90:(g + 1) * 90],
                                    identb[0:TCH, 0:TCH])
            nc.vector.tensor_copy(
                xfm[:, g, 0:363].rearrange("a (c t) -> a c t", c=3),
                xps[:, :, 0:TCH])
        xst = tiny.tile([90, 2, 3, 6], F32, tag="xst")
        for g in range(2):
            for c in range(3):
                nc.vector.bn_stats(
                    xst[:, g, c],
                    xfm[:, g, c * TCH:(c + 1) * TCH])
        xmv = tiny.tile([90, 2, 2], F32, tag="xmv")
        for g in range(2):
            nc.vector.bn_aggr(xmv[:, g], xst[:, g])
        d["xfm"], d["xmv"] = xfm, xmv
        hps = pa([96, 512])
        for g in range(2):
            nc.tensor.matmul(hps[:, 0:364], wproj[:, g, :], xfm[:, g],
                             start=(g == 0), stop=(g == 1))
        d["hps"] = hps

    def stage_hpad(s):
        """[G] gelu(proj + bias) into padded conv input row."""
        d = st[s]
        hpad = sb.tile([96, PAD + T + PAD + 5], F32R, tag="hpad")
        nc.vector.memset(hpad.bitcast(F32)[:, 0:PAD], 0.0)
        nc.vector.memset(hpad.bitcast(F32)[:, PAD + T:], 0.0)
        nc.scalar.activation(hpad[:, PAD:PAD + T], d["hps"][:, 0:T], AF.Gelu,
                             bias=bproj, scale=1.0)
        d["hpad"] = hpad
        if DEBUG and s < DBG_SAMPLES:
            dbg(f"h{s}", hpad.bitcast(F32)[:, PAD:PAD + T], (96, T))

    def stage_conv(s):
        """dw-folded-into-pw conv (no bias), cast to bf16, per-channel stats,
        group aggregation. Conv bias folds into the GN affine."""
        d = st[s]
        ysb = sb.tile([96, 3, 364], BF16, tag="ysb")
        for k in range(3):
            dd = DILS[k]
            yps = pa([96, 512])
            for j in range(7):
                off = PAD + (j - 3) * dd
                nc.tensor.matmul(yps[:, 0:364], wconv[k][:, j, :],
                                 d["hpad"][:, off:off + 364],
                                 start=(j == 0), stop=(j == 6))
            nc.vector.tensor_copy(ysb[:, k, 0:T], yps[:, 0:T])
        yst = tiny.tile([96, 3, 6], F32, tag="yst")
        for k in range(3):
            nc.vector.bn_stats(yst[:, k], ysb[:, k, 0:T])
        ymv = tiny.tile([96, 3, 2], F32, tag="ymv")
        for k in range(3):
            nc.vector.bn_aggr(ymv[:, k], yst[:, k])
        # st6: [mean + conv_bias, E[(y+b)^2]] per channel
        st6 = tiny.tile([96, 6], F32, tag="st6")
        nc.vector.tensor_tensor(st6[:, 0:3], ymv[:, :, 0], cb, op=ALU.add)
        nc.vector.tensor_tensor(st6[:, 3:6], st6[:, 0:3], st6[:, 0:3],
                                op=ALU.mult)
        nc.vector.tensor_tensor(st6[:, 3:6], st6[:, 3:6], ymv[:, :, 1],
                                op=ALU.add)
        gst_ps = pa([8, 6])
        nc.tensor.matmul(gst_ps, wgrp, st6, start=True, stop=True)
        gst = tiny.tile([8, 6], F32, tag="gst")
        nc.vector.tensor_copy(gst, gst_ps)
        gvar = tiny.tile([8, 3], F32, tag="gvar")
        nc.vector.tensor_tensor(gvar, gst[:, 0:3], gst[:, 0:3], op=ALU.mult)
        nc.vector.tensor_tensor(gvar, gst[:, 3:6], gvar, op=ALU.subtract)
        d["ysb"], d["gst"], d["gvar"] = ysb, gst, gvar

    def stage_gn_rsqrt(s):
        """[LnExp] group inv-std = exp(-0.5 ln(var+eps)); also corr channel
        inv-std from x stats."""
        d = st[s]
        gln = tiny.tile([8, 3], F32, tag="gln")
        nc.scalar.activation(gln, d["gvar"], AF.Ln, bias=epst[0:8, :], scale=1.0)
        nc.scalar.activation(d["gst"][:, 3:6], gln, AF.Exp, scale=-0.5)
        # corr: cinv = 1/max(sqrt(var*T/(T-1)), 1e-8)
        cvar = tiny.tile([90, 2], F32, tag="cvar")
        nc.vector.tensor_scalar_max(cvar, d["xmv"][:, :, 1], 1e-16)
        cln = tiny.tile([90, 2], F32, tag="cln")
        nc.scalar.activation(cln, cvar, AF.Ln, bias=0.0,
                             scale=float(T) / (T - 1))
        cinv = tiny.tile([90, 2], F32, tag="cinv")
        nc.scalar.activation(cinv, cln, AF.Exp, scale=-0.5)
        d["cinv"] = cinv

    def stage_gn_affine(s):
        """broadcast group stats to channels; per-channel scale/bias with the
        conv bias folded in: out = gelu(y*scl + bia),
        scl = g/sd_g, bia = gnb - (M_g - cb)*scl."""
        d = st[s]
        bc_ps = pa([96, 6])
        nc.tensor.matmul(bc_ps, wbc, d["gst"], start=True, stop=True)
        bc = tiny.tile([96, 6], F32, tag="bc")
        nc.vector.tensor_copy(bc, bc_ps)
        scl = tiny.tile([96, 3], F32, tag="scl")
        nc.vector.tensor_tensor(scl, gng, bc[:, 3:6], op=ALU.mult)
        bia = tiny.tile([96, 3], F32, tag="bia")
        nc.vector.tensor_tensor(bia, bc[:, 0:3], cb, op=ALU.subtract)
        nc.vector.tensor_tensor(bia, bia, scl, op=ALU.mult)
        nc.vector.tensor_tensor(bia, gnb, bia, op=ALU.subtract)
        d["scl"], d["bia"] = scl, bia

    def stage_cat(s):
        """[G] cat = gelu(GN(y)) in bf16."""
        d = st[s]
        cat = sb.tile([96, 3, 364], BF16, tag="cat")
        nc.vector.memset(cat[:, :, 363:364], 0.0)
        for k in range(3):
            nc.scalar.activation(cat[:, k, 0:T], d["ysb"][:, k, 0:T], AF.Gelu,
                                 bias=d["bia"][:, k:k + 1],
                                 scale=d["scl"][:, k:k + 1])
        d["cat"] = cat
        if DEBUG and s < DBG_SAMPLES:
            dbg(f"cat{s}", cat[:, :, 0:T], (96, 3 * T))

    def stage_merge(s):
        """merge matmul, bias, transpose to token-major, LN stats."""
        d = st[s]
        ups = pa([128, 512])
        for g in range(3):
            nc.tensor.matmul(ups[:, 0:364], mw[g], d["cat"][:, g],
                             start=(g == 0), stop=(g == 2))
        ufm = sb.tile([128, 364], BF16, tag="ufm")
        nc.scalar.activation(ufm[:, 0:T], ups[:, 0:T], AF.Identity,
                             bias=mb, scale=1.0)
        tpm = pa([TCH, 3, 128], BF16)
        for c in range(3):
            nc.tensor.transpose(tpm[:, c, :],
                                ufm[:, c * TCH:(c + 1) * TCH], identb)
        mst = tiny.tile([TCH, 3, 6], F32, tag="mst")
        for c in range(3):
            nc.vector.bn_stats(mst[:, c], tpm[:, c])
        mmv = tiny.tile([TCH, 3, 2], F32, tag="mmv")
        for c in range(3):
            nc.vector.bn_aggr(mmv[:, c], mst[:, c])
        d["tpm"], d["mmv"] = tpm, mmv

    def stage_mln_rsqrt(s):
        """[LnExp] token inv-std for merge LN."""
        d = st[s]
        mln_t = tiny.tile([TCH, 3], F32, tag="mln_t")
        nc.scalar.activation(mln_t, d["mmv"][:, :, 1], AF.Ln,
                             bias=epst[0:TCH, :], scale=1.0)
        mrs = tiny.tile([TCH, 3], F32, tag="mrs")
        nc.scalar.activation(mrs, mln_t, AF.Exp, scale=-0.5)
        d["mrs"] = mrs

    def stage_h0(s):
        """normalize, transpose back, [G] h0 = gelu(hat*g+b) in bf16."""
        d = st[s]
        hatm = sb.tile([TCH, 3, 128], BF16, tag="hatm")
        for c in range(3):
            nc.vector.tensor_scalar(hatm[:, c], d["tpm"][:, c],
                                    d["mmv"][:, c, 0:1], d["mrs"][:, c:c + 1],
                                    op0=ALU.subtract, op1=ALU.mult)
        tp2 = pa([128, 3, 128], BF16)
        for c in range(3):
            nc.tensor.transpose(tp2[:, c, 0:TCH], hatm[:, c, :],
                                identb[0:TCH, 0:TCH])
        h0 = sb.tile([128, 364], BF16, tag="h0")
        nc.vector.memset(h0[:, 363:364], 0.0)
        nc.scalar.activation(
            h0[:, 0:363].rearrange("a (c t) -> a c t", c=3), tp2[:, :, 0:TCH],
            AF.Gelu, bias=mlnb, scale=mlng)
        d["h0"] = h0
        if DEBUG and s < DBG_SAMPLES:
            dbg(f"h0_{s}", h0[:, 0:T], (128, T))

    def ln_tm(s, src_fm, pfx, keep_hat=False):
        """transpose fm->tm, stats, [LnExp] inv-std, normalize (f32r hat)."""
        d = st[s]
        tp = pa([TCH, 3, 128], BF16)
        for c in range(3):
            nc.tensor.transpose(tp[:, c, :],
                                src_fm[:, c * TCH:(c + 1) * TCH], identb)
        lst = tiny.tile([TCH, 3, 6], F32, tag=f"{pfx}st")
        for c in range(3):
            nc.vector.bn_stats(lst[:, c], tp[:, c])
        lmv = tiny.tile([TCH, 3, 2], F32, tag=f"{pfx}mv")
        for c in range(3):
            nc.vector.bn_aggr(lmv[:, c], lst[:, c])
        lln = tiny.tile([TCH, 3], F32, tag=f"{pfx}ln")
        nc.scalar.activation(lln, lmv[:, :, 1], AF.Ln,
                             bias=epst[0:TCH, :], scale=1.0)
        lrs = tiny.tile([TCH, 3], F32, tag=f"{pfx}rs")
        nc.scalar.activation(lrs, lln, AF.Exp, scale=-0.5)
        pool_ = p2 if keep_hat else sb
        hat = pool_.tile([TCH, 3, 128], BF16, tag=f"{pfx}hat")
        for c in range(3):
            nc.vector.tensor_scalar(hat[:, c], tp[:, c],
                                    lmv[:, c, 0:1], lrs[:, c:c + 1],
                                    op0=ALU.subtract, op1=ALU.mult)
        tpb = pa([128, 3, 128], BF16)
        for c in range(3):
            nc.tensor.transpose(tpb[:, c, 0:TCH], hat[:, c, :],
                                identb[0:TCH, 0:TCH])
        return hat, tpb

    def stage_p2(s):
        """[LnExp table throughout] qkv -> attention -> out-proj -> ln1 ->
        ffn -> ln2 -> attentive pooling -> correlation gram."""
        d = st[s]
        h0 = d["h0"]
        if P2STOP == 0:
            return
        # ---- qkv ----
        qfm = p2.tile([64, 2, 364], BF16, tag="qfm")
        kfm = p2.tile([64, 2, 364], BF16, tag="kfm")
        vfm = p2.tile([128, 364], BF16, tag="vfm")
        nc.vector.memset(qfm[:, :, 363:364], 0.0)
        nc.vector.memset(kfm[:, :, 363:364], 0.0)
        qps = pa([128, 512])
        nc.tensor.matmul(qps[:, 0:364], qkvT[0], h0, start=True, stop=True)
        for i in range(2):
            nc.scalar.activation(qfm[:, i, 0:T],
                                 qps[i * 64:(i + 1) * 64, 0:T],
                                 AF.Identity, bias=qb_s[i * 64:(i + 1) * 64, :],
                                 scale=ISQ)
        kps = pa([128, 512])
        nc.tensor.matmul(kps[:, 0:364], qkvT[1], h0, start=True, stop=True)
        for i in range(2):
            nc.vector.tensor_scalar(kfm[:, i, 0:T],
                                    kps[i * 64:(i + 1) * 64, 0:T],
                                    1.0, qb3[i * 64:(i + 1) * 64, 1:2],
                                    op0=ALU.mult, op1=ALU.add)
        vps = pa([128, 512])
        nc.tensor.matmul(vps[:, 0:364], qkvT[2], h0, start=True, stop=True)
        nc.vector.tensor_scalar(vfm[:, 0:T], vps[:, 0:T], 1.0, qb3[:, 2:3],
                                op0=ALU.mult, op1=ALU.add)
        vtp = pa([TCH, 3, 128], BF16)
        for c in range(3):
            nc.tensor.transpose(vtp[:, c, :], vfm[:, c * TCH:(c + 1) * TCH],
                                identb)
        vtm = p2.tile([TCH, 3, 4, 33], BF16, tag="vtm")
        nc.vector.tensor_copy(vtm[:, :, :, 0:32],
                              vtp.rearrange("p c (h d) -> p c h d", h=4))
        nc.vector.memset(vtm[:, :, :, 32:33], 1.0)

        if P2STOP == 1:
            return
        # ---- scores (transposed) + exp (bf16) ----
        expt = p2.tile([TCH, 3, 2, 2, 364], BF16, tag="expt")
        for cs in range(3):
            for b_ in range(2):
                scps = pa([TCH, 2, 512])
                for i in range(2):
                    nc.tensor.matmul(
                        scps[:, i, 0:364],
                        kfm[b_ * 32:(b_ + 1) * 32, i,
                            cs * TCH:(cs + 1) * TCH],
                        qfm[b_ * 32:(b_ + 1) * 32, i, :],
                        start=True, stop=True)
                nc.scalar.activation(
                    expt[:, cs, b_], scps[:, :, 0:364], AF.Exp)

        if P2STOP == 2:
            return
        # ---- AV: V stationary (33-col LDW), exp moving N=364; out is
        # feature-major per-head [33,364] with Z on row 32; heads packed two
        # per 1-bank psum tile at partition offsets 0/64. ----
        avps = [pa([97, 512]), pa([97, 512])]
        for h in range(4):
            tile_ = avps[h // 2]
            po = (h % 2) * 64
            for cs in range(3):
                nc.tensor.matmul(
                    tile_[po:po + 33, 0:364],
                    vtm[:, cs, h, :],
                    expt[:, cs, h % 2, h // 2, :],
                    start=(cs == 0), stop=(cs == 2))
        # 1/Z rows -> one sbuf row-tile (free-dim stacked), PE-broadcast to
        # 32 partitions per head, then normalize O per head.
        zln = p2.tile([1, 4, 364], F32, tag="zln")
        for h in range(4):
            nc.scalar.activation(
                zln[:, h, :],
                avps[h // 2][(h % 2) * 64 + 32:(h % 2) * 64 + 33, 0:364],
                AF.Ln)
        rzrow = p2.tile([1, 4, 364], BF16, tag="rzrow")
        nc.scalar.activation(rzrow, zln, AF.Exp, scale=-1.0)
        rzb = pa([97, 2, 512])
        for h in range(4):
            nc.tensor.matmul(rzb[(h % 2) * 64:(h % 2) * 64 + 32, h // 2, 0:364],
                             ones32, rzrow[:, h, :], start=True, stop=True)
        rzbs = p2.tile([97, 2, 364], BF16, tag="rzbs")
        nc.vector.tensor_copy(rzbs[0:32], rzb[0:32, :, 0:364])
        nc.vector.tensor_copy(rzbs[64:96], rzb[64:96, :, 0:364])
        oat = p2.tile([97, 2, 364], BF16, tag="oat")
        nc.vector.memset(oat, 0.0)
        for h in range(4):
            po = (h % 2) * 64
            nc.vector.tensor_tensor(oat[po:po + 32, h // 2, :],
                                    avps[h // 2][po:po + 32, 0:364],
                                    rzbs[po:po + 32, h // 2, :], op=ALU.mult)

        if P2STOP == 3:
            return
        # ---- out proj (4 per-head accumulating matmuls) + residual ----
        rps = pa([128, 512])
        for p_ in range(2):
            nc.tensor.matmul(rps[:, 0:364], owT_s[p_], oat[:, p_, :],
                             start=(p_ == 0), stop=(p_ == 1))
        if P2STOP == 31:
            return
        rfm = p2.tile([128, 364], BF16, tag="rfm")
        nc.vector.scalar_tensor_tensor(rfm[:, 0:T], rps[:, 0:T], ob,
                                       h0[:, 0:T], op0=ALU.add, op1=ALU.add)
        if P2STOP == 32:
            return
        if DEBUG and s < DBG_SAMPLES:
            dbg(f"rfm{s}", rfm[:, 0:T], (128, T))
        hat1, tpb1 = ln_tm(s, rfm, "l1", keep_hat=False)
        if P2STOP == 33:
            return
        h1 = p2.tile([128, 364], BF16, tag="h1")
        nc.vector.memset(h1[:, 363:364], 0.0)
        nc.scalar.activation(
            h1[:, 0:363].rearrange("a (c t) -> a c t", c=3), tpb1[:, :, 0:TCH],
            AF.Identity, bias=ln1b, scale=ln1g)

        if P2STOP == 4:
            return
        # ---- ffn ----
        g1 = p2.tile([128, 2, 364], BF16, tag="g1")
        nc.vector.memset(g1[:, :, 363:364], 0.0)
        for i in range(2):
            f1ps = pa([128, 512])
            nc.tensor.matmul(f1ps[:, 0:364], f1T[i], h1,
                             start=True, stop=True)
            nc.scalar.activation(g1[:, i, 0:T], f1ps[:, 0:T], AF.Relu,
                                 bias=f1b[:, i:i + 1], scale=1.0)
        f2ps = pa([128, 512])
        for i in range(2):
            nc.tensor.matmul(f2ps[:, 0:364], f2T[i], g1[:, i],
                             start=(i == 0), stop=(i == 1))
        ffo = p2.tile([128, 364], BF16, tag="ffo")
        nc.vector.scalar_tensor_tensor(ffo[:, 0:T], f2ps[:, 0:T], f2b,
                                       h1[:, 0:T], op0=ALU.add, op1=ALU.add)
        hat2, tpb2 = ln_tm(s, ffo, "l2", keep_hat=True)
        h2 = p2.tile([128, 364], BF16, tag="h2")
        nc.vector.memset(h2[:, 363:364], 0.0)
        nc.scalar.activation(
            h2[:, 0:363].rearrange("a (c t) -> a c t", c=3), tpb2[:, :, 0:TCH],
            AF.Identity, bias=ln2b, scale=ln2g)
        if DEBUG and s < DBG_SAMPLES:
            dbg(f"h2_{s}", h2[:, 0:T], (128, T))

        if P2STOP == 5:
            return
        # ---- attentive pooling ----
        plps = pa([1, 512])
        nc.tensor.matmul(plps[:, 0:364], poolw, h2, start=True, stop=True)
        pw_sb = p2.tile([1, T], F32, tag="pw_sb")
        zp = tiny.tile([1, 1], F32, tag="zp")
        nc.scalar.activation(pw_sb, plps[:, 0:T], AF.Exp,
                             bias=poolb, scale=1.0, accum_out=zp)
        rzp = tiny.tile([1, 1], F32, tag="rzp")
        nc.vector.reciprocal(rzp, zp)
        wn = p2.tile([1, T], BF16, tag="wn")
        nc.vector.tensor_scalar_mul(wn, pw_sb, rzp)
        wtp = pa([TCH, 3, 2], BF16)
        for c in range(3):
            nc.tensor.transpose(wtp[:, c, 0:1],
                                wn[:, c * TCH:(c + 1) * TCH],
                                identb[0:1, 0:1])
        wcol = tiny.tile([TCH, 3, 1], BF16, tag="wcol")
        nc.vector.tensor_copy(wcol, wtp[:, :, 0:1])
        tps = pa([128, 1])
        for c in range(3):
            nc.tensor.matmul(tps, hat2[:, c, :], wcol[:, c, :],
                             start=(c == 0), stop=(c == 2))
        nc.vector.tensor_scalar(fus_t[:, s:s + 1], tps, ln2g, ln2b,
                                op0=ALU.mult, op1=ALU.add)

        if P2STOP == 6:
            return
        # ---- correlation fingerprint ----
        wcorr = p2.tile([90, 2, 12], BF16, tag="wcorr")
        for g in range(2):
            nc.vector.tensor_scalar_mul(wcorr[:, g], cmask[:, g],
                                        d["cinv"][:, g:g + 1])
        swps = pa([12, 512])
        for g in range(2):
            nc.tensor.matmul(swps[:, 0:364], wcorr[:, g], d["xfm"][:, g],
                             start=(g == 0), stop=(g == 1))
        swsb = p2.tile([12, T], BF16, tag="swsb")
        rsum = tiny.tile([12, 1], F32, tag="rsum")
        nc.vector.tensor_scalar(swsb, swps[:, 0:T], 1.0, 0.0, op0=ALU.mult,
                                op1=ALU.add, accum_out=rsum)
        swtp = pa([TCH, 3, 12], BF16)
        for c in range(3):
            nc.tensor.transpose(swtp[:, c, :], swsb[:, c * TCH:(c + 1) * TCH],
                                identb[0:12, 0:12])
        swtm = tiny.tile([TCH, 3, 12], BF16, tag="swtm")
        nc.vector.tensor_copy(swtm, swtp)
        rsT_ps = pa([1, 12])
        nc.tensor.transpose(rsT_ps, rsum, ident[0:12, 0:12])
        rsT = tiny.tile([1, 12], F32, tag="rsT")
        nc.vector.tensor_copy(rsT, rsT_ps)
        rsTn = tiny.tile([1, 12], F32, tag="rsTn")
        nc.vector.tensor_scalar_mul(rsTn, rsT, -1.0 / T)
        gps = pa([12, 12])
        for c in range(3):
            nc.tensor.matmul(gps, swtm[:, c, :], swtm[:, c, :],
                             start=(c == 0), stop=False)
        nc.tensor.matmul(gps, rsTn, rsT, start=False, stop=True)
        nc.vector.tensor_copy(bm_all[:, s, :], gps)

    # ================= block-scheduled emission =================
    for b0 in range(0, S, BLK):
        blk = range(b0, min(b0 + BLK, S))
        for s in blk:
            stage_load(s)
        for s in blk:
            stage_hpad(s)       # [G]
        for s in blk:
            stage_conv(s)
        for s in blk:
            stage_gn_rsqrt(s)   # [LnExp]
        for s in blk:
            stage_gn_affine(s)
        for s in blk:
            stage_cat(s)        # [G]
        for s in blk:
            stage_merge(s)
        for s in blk:
            stage_mln_rsqrt(s)  # [LnExp]
        for s in blk:
            stage_h0(s)         # [G]
        for s in blk:
            stage_p2(s)         # [LnExp]

    # ================= batched tail =================
    bm_dram = nc.dram_tensor("bm_scratch", [12, S, 12], F32).ap()
    dma.dma_start(out=bm_dram, in_=bm_all)
    fcv = W.tile([78, S], F32, tag="fcv")
    row_off = 0
    for i in range(12):
        n = 12 - i
        dma.dma_start(
            out=fcv[row_off:row_off + n, :],
            in_=bm_dram[i, :, i:12].transpose([1, 0]))
        row_off += n
    fcps = pa([64, S])
    nc.tensor.matmul(fcps, fcwk, fcv, start=True, stop=True)
    nc.scalar.activation(fus_f, fcps, AF.Gelu, bias=fcb, scale=1.0)

    fu_ps = pa([128, S])
    nc.tensor.matmul(fu_ps, fu1T[:, 0, :], fus_t, start=True, stop=False)
    nc.tensor.matmul(fu_ps, fu1T[0:64, 1, :], fus_f, start=False, stop=True)
    zfm = W.tile([128, S], F32, tag="zfm")
    nc.vector.tensor_scalar(zfm, fu_ps, 1.0, fu1b, op0=ALU.mult, op1=ALU.add)
    ztp = pa([S, 128])
    nc.tensor.transpose(ztp, zfm, ident)
    ztm = W.tile([S, 128], F32, tag="ztm")
    nc.vector.tensor_copy(ztm, ztp)
    zst = W.tile([S, 6], F32, tag="zst")
    nc.vector.bn_stats(zst, ztm)
    zmv = W.tile([S, 2], F32, tag="zmv")
    nc.vector.bn_aggr(zmv, zst)
    zln = W.tile([S, 1], F32, tag="zln")
    nc.scalar.activation(zln, zmv[:, 1:2], AF.Ln, bias=epst[0:S, :], scale=1.0)
    zrs = W.tile([S, 1], F32, tag="zrs")
    nc.scalar.activation(zrs, zln, AF.Exp, scale=-0.5)
    zhat = W.tile([S, 128], F32, tag="zhat")
    nc.vector.tensor_scalar(zhat, ztm, zmv[:, 0:1], zrs,
                            op0=ALU.subtract, op1=ALU.mult)
    zhtp = pa([128, S])
    nc.tensor.transpose(zhtp, zhat, ident[0:S, 0:S])
    zg = W.tile([128, S], F32, tag="zg")
    nc.scalar.activation(zg, zhtp, AF.Gelu, bias=flb, scale=flg)
    out_ps = pa([64, S])
    nc.tensor.matmul(out_ps, fu2T, zg, start=True, stop=True)
    out_sb = W.tile([64, S], F32, tag="out_sb")
    nc.scalar.activation(out_sb, out_ps, AF.Identity, bias=fu2b, scale=1.0)
    outT_ps = pa([S, 64])
    nc.tensor.transpose(outT_ps, out_sb, ident[0:64, 0:64])
    outT = W.tile([S, 64], F32, tag="outT")
    nc.vector.tensor_copy(outT, outT_ps)
    dma.dma_start(out=out_dram, in_=outT)

    for p in reversed(pools):
        p.__exit__(None, None, None)


_PROGRAM = None


def _get_program():
    global _PROGRAM
    if _PROGRAM is None:
        _PROGRAM = build_program()
    return _PROGRAM


def kernel(**inputs):
    from concourse.bass_utils import run_bass_kernel_spmd

    nc, _ = _get_program()
    in_maps = []
    for c in range(NCORES):
        m = {}
        for name, _shape in INPUT_SPECS:
            if name == "x":
                m["x"] = np.ascontiguousarray(
                    np.asarray(inputs["x"][c * S:(c + 1) * S], dtype=np.float32))
            else:
                m[name] = np.ascontiguousarray(
                    np.asarray(inputs[name], dtype=np.float32))
        in_maps.append(m)
    res = run_bass_kernel_spmd(nc, in_maps, list(range(NCORES)))
    global LAST_RESULTS
    LAST_RESULTS = res
    out = np.concatenate([res.results[c]["out"] for c in range(NCORES)], axis=0)
    return out.astype(np.float32)


LAST_RESULTS = None


# revision 25
# speedup vs baseline: 1.1767x; 1.0383x over previous
"""Trainium2 Bass kernel for nn_NetworkAwareClassicalExpert (B=256,T=363,C=180).

Data-parallel over 8 NeuronCores: 32 samples/core. Per-core program processes
samples feature-major (channels-on-partitions), transposing to token-major for
layernorm/softmax row ops.

v2 (fast) design notes:
  - bf16 for all per-token matmuls (merge/qkv/scores/AV/out/ffn/pool);
    f32r for proj/conv/corr (N>=256 streams at 1 cycle/row either way, and
    LDWEIGHTS overlaps the previous matmul's stream, so dtype only matters
    for accuracy + small-N ops).
  - all PE transposes in f32r (1.5 c/r, single LDW pass); transpose outputs
    stay fp32 in PSUM so vector/scalar psum reads are plain f32.
  - every rsqrt is exp(-0.5*ln(v+eps)) on the scalar engine: ln/exp live in
    the same activation table as softmax's exp, so the only table switches
    left are gelu<->ln/exp.
  - samples emitted in blocks of 4 with stage-major ordering inside the
    block, so the scalar queue sees [gelu x4][ln/exp x4][gelu x12]... =
    ~6 table loads per block instead of ~8 per sample.
  - conv bias folded into the GroupNorm affine (mean shifts by the bias,
    per-channel variance is unchanged), saving the ones-row bias matmuls.
  - attention: scores computed transposed [s,t], exp via ACT -> bf16; AV as
    N=33 bf16 matmuls (32ns each) with an appended ones-column giving the
    softmax normalizer Z for free.
  - FC fingerprint: block means of the 180x180 correlation collapse to a
    12x12 Gram of per-network invstd-weighted channel sums + rank-1 mean
    correction.
"""

import sys
import os

sys.path.insert(0, "/opt/trn_rl_repo")

import numpy as np

import concourse.bass as bass
import concourse.mybir as mybir
import concourse.tile as tile
import bass_rust
from concourse.vector_clock import ScopedClock
from concourse.masks import make_identity

F32 = mybir.dt.float32
F32R = mybir.dt.float32r
BF16 = mybir.dt.bfloat16
AF = mybir.ActivationFunctionType
ALU = mybir.AluOpType

B, T, C = 256, 363, 180
CD = 96
DM = 128
DILS = (1, 4, 16)
NCORES = 8
S = int(os.environ.get("KB_NSAMP", str(B // NCORES)))
TCH = 121                # t-chunk (3 chunks of 121)
PAD = 48
EPS = 1e-5
ISQ = float(1.0 / np.sqrt(32.0))
BLK = 4

DEBUG = bool(int(os.environ.get("KBDBG", "0")))
DBG_SAMPLES = int(os.environ.get("KBDBG_S", "2"))


def _patch_tile_drain():
    """This walrus rejects >1 sem wait on the final Tile drain: split them."""

    def _drain_and_barrier(self, tick_clock, wait_clock):
        drain_inst = self.nc.sync.drain()
        wait_clock.add_sem_waits(
            drain_inst.ins, ScopedClock({None: tick_clock.global_clock})
        )
        si = drain_inst.ins.sync_info
        if si is not None and si.on_wait is not None and len(si.on_wait) > 1:
            waits = list(si.on_wait)
            ups = list(si.on_update) if si.on_update else []
            drain_inst.ins.sync_info = bass_rust.SyncInfo(
                on_wait=waits[:1], on_update=ups
            )
            for w in waits[1:]:
                nop = self.nc.sync.nop()
                nop.ins.sync_info = bass_rust.SyncInfo(on_wait=[w], on_update=[])
        self.nc.all_engine_barrier()
        popped = self.nc._tile_sem_poison_stack.pop()
        assert popped is self._sem_poison
        if not int(os.environ.get("KB_NOSEMCLEAR", "0")):
            self.nc.clear_and_free_semaphores(list(self.sems.allocated().values()))
        self.nc.all_engine_barrier()

    tile.TileContext._drain_and_barrier = _drain_and_barrier


_patch_tile_drain()


def nn_cur_bb(nc):
    bbw = nc.cur_bb
    return bbw.bb if hasattr(bbw, "bb") else bbw


def _split_sync_waits(nc, max_waits=1):
    """walrus rejects instructions with >1 sem wait; hoist excess onto
    same-engine NOPs inserted immediately before."""
    for f in nc.m.functions:
        for bb in f.blocks:
            insts = list(bb.instructions)
            out = []
            changed = False
            for inst in insts:
                si = getattr(inst, "sync_info", None)
                if si is not None and si.on_wait and len(si.on_wait) > max_waits:
                    waits = list(si.on_wait)
                    ups = list(si.on_update) if si.on_update else []
                    extra = waits[max_waits:]
                    for i in range(0, len(extra), max_waits):
                        nop = nc.engines[inst.engine].nop(nofuse=True)
                        cur = nn_cur_bb(nc)
                        lst = list(cur.instructions)
                        assert lst and lst[-1].name == nop.ins.name
                        cur.instructions = lst[:-1]
                        nop.ins.sync_info = bass_rust.SyncInfo(
                            on_wait=extra[i:i + max_waits], on_update=[])
                        out.append(nop.ins)
                    inst.sync_info = bass_rust.SyncInfo(
                        on_wait=waits[:max_waits], on_update=ups)
                    changed = True
                out.append(inst)
            if changed:
                bb.instructions = out


INPUT_SPECS = [
    ("x", (S, T, C)),
    ("w_proj", (12, 15, 8)), ("b_proj", (12, 8)),
    ("dw_w", (3, 96, 7)), ("dw_b", (3, 96)),
    ("pw_w", (3, 96, 96)), ("pw_b", (3, 96)),
    ("gn_g", (3, 96)), ("gn_b", (3, 96)),
    ("merge_w", (288, 128)), ("merge_b", (128,)),
    ("merge_ln_g", (128,)), ("merge_ln_b", (128,)),
    ("qkv_w", (384, 128)), ("qkv_b", (384,)),
    ("out_w", (128, 128)), ("out_b", (128,)),
    ("ln1_g", (128,)), ("ln1_b", (128,)),
    ("ff1_w", (256, 128)), ("ff1_b", (256,)),
    ("ff2_w", (128, 256)), ("ff2_b", (128,)),
    ("ln2_g", (128,)), ("ln2_b", (128,)),
    ("pool_w", (128, 1)), ("pool_b", (1,)),
    ("fc_w", (78, 64)), ("fc_b", (64,)),
    ("fus1_w", (192, 128)), ("fus1_b", (128,)),
    ("fus_ln_g", (128,)), ("fus_ln_b", (128,)),
    ("fus2_w", (128, 64)), ("fus2_b", (64,)),
]


def build_program():
    nc = bass.Bass("TRN2", target_bir_lowering=False, debug=False,
                   num_devices=NCORES)
    D = {}
    for name, shape in INPUT_SPECS:
        D[name] = nc.dram_tensor(name, list(shape), F32, kind="ExternalInput").ap()
    out_dram = nc.dram_tensor("out", [S, 64], F32, kind="ExternalOutput").ap()
    dbg_shapes = {}

    with tile.TileContext(nc) as tc:
        with nc.allow_low_precision(reason="deliberate bf16/f32r pipeline"):
            _build(nc, tc, D, out_dram, dbg_shapes)
    if not int(os.environ.get("KB_NOSPLIT", "0")):
        _split_sync_waits(nc)
    return nc, dbg_shapes


def _build(nc, tc, D, out_dram, dbg_shapes):
    pools = []

    def mkpool(name, bufs, space="SBUF"):
        p = tc.tile_pool(name=name, bufs=bufs, space=space)
        pools.append(p)
        return p.__enter__()

    W = mkpool("weights", 1)        # persistent tiles, one tag each
    sb = mkpool("sb", 6)            # per-sample P1 state (block depth 4 + lag)
    p2 = mkpool("p2", 2)            # per-sample P2 transients
    tiny = mkpool("tiny", 8)        # small per-sample stats
    pp = mkpool("pp", 4, "PSUM")    # unified psum pool: 4 x 4KB buffers

    dma = nc.sync

    _pa_n = [0]

    def pa(shape, dtype=F32):
        _pa_n[0] += 1
        return pp.tile(list(shape), dtype, tag="a", name=f"pa{_pa_n[0]}")

    def dbg(name, ap, shape):
        if not DEBUG:
            return
        t = nc.dram_tensor(f"dbg_{name}", list(shape), F32,
                           kind="ExternalOutput").ap()
        dbg_shapes[name] = tuple(shape)
        if ap.dtype != F32:
            tmp = sb.tile(list(shape), F32, tag=f"dbgt_{name}")
            nc.vector.tensor_copy(tmp, ap)
            dma.dma_start(out=t, in_=tmp)
        else:
            dma.dma_start(out=t, in_=ap)

    # ================= weight preload =================
    ident = W.tile([128, 128], F32, tag="ident")
    make_identity(nc, ident)
    identb = W.tile([128, 128], BF16, tag="identb")
    nc.vector.tensor_copy(identb, ident)
    ones32 = W.tile([1, 32], BF16, tag="ones32")
    nc.vector.memset(ones32, 1.0)

    wprojf = W.tile([90, 2, 96], F32, tag="wprojf")
    nc.vector.memset(wprojf, 0.0)
    for n in range(12):
        g, j = divmod(n, 6)
        dma.dma_start(out=wprojf[j * 15:(j + 1) * 15, g, n * 8:(n + 1) * 8],
                      in_=D["w_proj"][n])
    wproj = W.tile([90, 2, 96], BF16, tag="wproj")
    nc.vector.tensor_copy(wproj, wprojf)
    bproj = W.tile([96, 1], F32, tag="bproj")
    dma.dma_start(out=bproj, in_=D["b_proj"].rearrange("a b -> (a b)").unsqueeze(1))

    pwT, dwk = [], []
    for k in range(3):
        t_ = W.tile([96, 96], F32, tag=f"pwT{k}")
        dma.dma_start(out=t_, in_=D["pw_w"][k].transpose([1, 0]))
        pwT.append(t_)
        t2 = W.tile([96, 7], F32, tag=f"dw{k}")
        dma.dma_start(out=t2, in_=D["dw_w"][k])
        dwk.append(t2)
    wconv = []
    for k in range(3):
        t_ = W.tile([96, 7, 96], F32R, tag=f"wconv{k}")
        for j in range(7):
            nc.vector.tensor_scalar_mul(t_[:, j, :], pwT[k], dwk[k][:, j:j + 1])
        wconv.append(t_)
    dwb = W.tile([96, 3], F32, tag="dwb")
    dma.dma_start(out=dwb, in_=D["dw_b"].transpose([1, 0]))
    pwb = W.tile([96, 3], F32, tag="pwb")
    dma.dma_start(out=pwb, in_=D["pw_b"].transpose([1, 0]))
    cb_ps = pa([96, 3])
    for k in range(3):
        nc.tensor.matmul(cb_ps[:, k:k + 1], pwT[k], dwb[:, k:k + 1],
                         start=True, stop=True, skip_group_check=True)
    cb = W.tile([96, 3], F32, tag="cb")
    nc.vector.tensor_add(cb, cb_ps, pwb)

    gng = W.tile([96, 3], F32, tag="gng")
    dma.dma_start(out=gng, in_=D["gn_g"].transpose([1, 0]))
    gnb = W.tile([96, 3], F32, tag="gnb")
    dma.dma_start(out=gnb, in_=D["gn_b"].transpose([1, 0]))

    # wgrp[c, g] = 1/12 iff 0 <= c - 12g <= 11 ; wbc[g, c] = 1 iff same
    wgrp = W.tile([96, 8], F32, tag="wgrp")
    nc.vector.memset(wgrp, 1.0 / 12.0)
    nc.gpsimd.affine_select(out=wgrp, in_=wgrp, compare_op=ALU.is_ge,
                            fill=0.0, base=0, pattern=[[-12, 8]],
                            channel_multiplier=1)
    nc.gpsimd.affine_select(out=wgrp, in_=wgrp, compare_op=ALU.is_ge,
                            fill=0.0, base=11, pattern=[[12, 8]],
                            channel_multiplier=-1)
    wbc = W.tile([8, 96], F32, tag="wbc")
    nc.vector.memset(wbc, 1.0)
    nc.gpsimd.affine_select(out=wbc, in_=wbc, compare_op=ALU.is_ge,
                            fill=0.0, base=0, pattern=[[1, 96]],
                            channel_multiplier=-12)
    nc.gpsimd.affine_select(out=wbc, in_=wbc, compare_op=ALU.is_ge,
                            fill=0.0, base=11, pattern=[[-1, 96]],
                            channel_multiplier=12)

    wst = W.tile([128, 128], F32, tag="wst")  # staging for bf16 casts

    mw = []
    for g in range(3):
        t_ = W.tile([96, 128], BF16, tag=f"mw{g}")
        dma.dma_start(out=wst[0:96, :], in_=D["merge_w"][g * 96:(g + 1) * 96, :])
        nc.vector.tensor_copy(t_, wst[0:96, :])
        mw.append(t_)
    mb = W.tile([128, 1], F32, tag="mb")
    dma.dma_start(out=mb, in_=D["merge_b"].unsqueeze(1))
    mlng = W.tile([128, 1], F32, tag="mlng")
    dma.dma_start(out=mlng, in_=D["merge_ln_g"].unsqueeze(1))
    mlnb = W.tile([128, 1], F32, tag="mlnb")
    dma.dma_start(out=mlnb, in_=D["merge_ln_b"].unsqueeze(1))

    qkvT = []
    for i in range(3):
        t_ = W.tile([128, 128], BF16, tag=f"qkvT{i}")
        dma.dma_start(out=wst,
                      in_=D["qkv_w"][i * 128:(i + 1) * 128, :].transpose([1, 0]))
        nc.vector.tensor_copy(t_, wst)
        qkvT.append(t_)
    qb3 = W.tile([128, 3], F32, tag="qb3")
    dma.dma_start(out=qb3, in_=D["qkv_b"].rearrange("(a b) -> b a", a=3))
    qb_s = W.tile([128, 1], F32, tag="qb_s")
    nc.vector.tensor_scalar_mul(qb_s, qb3[:, 0:1], ISQ)

    owT_s = []
    dma.dma_start(out=wst, in_=D["out_w"].transpose([1, 0]))
    for p in range(2):
        t_ = W.tile([97, 128], BF16, tag=f"owTs{p}")
        nc.vector.memset(t_, 0.0)
        nc.vector.tensor_copy(t_[0:32, :], wst[p * 64:p * 64 + 32, :])
        nc.vector.tensor_copy(t_[64:96, :], wst[p * 64 + 32:p * 64 + 64, :])
        owT_s.append(t_)
    ob = W.tile([128, 1], F32, tag="ob")
    dma.dma_start(out=ob, in_=D["out_b"].unsqueeze(1))

    ln1g = W.tile([128, 1], F32, tag="ln1g")
    dma.dma_start(out=ln1g, in_=D["ln1_g"].unsqueeze(1))
    ln1b = W.tile([128, 1], F32, tag="ln1b")
    dma.dma_start(out=ln1b, in_=D["ln1_b"].unsqueeze(1))
    ln2g = W.tile([128, 1], F32, tag="ln2g")
    dma.dma_start(out=ln2g, in_=D["ln2_g"].unsqueeze(1))
    ln2b = W.tile([128, 1], F32, tag="ln2b")
    dma.dma_start(out=ln2b, in_=D["ln2_b"].unsqueeze(1))

    f1T, f2T = [], []
    for i in range(2):
        t_ = W.tile([128, 128], BF16, tag=f"f1T{i}")
        dma.dma_start(out=wst,
                      in_=D["ff1_w"][i * 128:(i + 1) * 128, :].transpose([1, 0]))
        nc.vector.tensor_copy(t_, wst)
        f1T.append(t_)
        t2 = W.tile([128, 128], BF16, tag=f"f2T{i}")
        dma.dma_start(out=wst,
                      in_=D["ff2_w"][:, i * 128:(i + 1) * 128].transpose([1, 0]))
        nc.vector.tensor_copy(t2, wst)
        f2T.append(t2)
    f1b = W.tile([128, 2], F32, tag="f1b")
    dma.dma_start(out=f1b, in_=D["ff1_b"].rearrange("(a b) -> b a", a=2))
    f2b = W.tile([128, 1], F32, tag="f2b")
    dma.dma_start(out=f2b, in_=D["ff2_b"].unsqueeze(1))

    poolw = W.tile([128, 1], BF16, tag="poolw")
    dma.dma_start(out=wst[:, 0:1], in_=D["pool_w"])
    nc.vector.tensor_copy(poolw, wst[:, 0:1])
    poolb = W.tile([1, 1], F32, tag="poolb")
    dma.dma_start(out=poolb, in_=D["pool_b"].unsqueeze(1))

    # cmask[p, g, n] = 1 iff 0 <= p - 15*(n - 6g) <= 14
    cmask = W.tile([90, 2, 12], F32, tag="cmask")
    nc.vector.memset(cmask, 1.0)
    nc.gpsimd.affine_select(out=cmask, in_=cmask, compare_op=ALU.is_ge,
                            fill=0.0, base=0, pattern=[[90, 2], [-15, 12]],
                            channel_multiplier=1)
    nc.gpsimd.affine_select(out=cmask, in_=cmask, compare_op=ALU.is_ge,
                            fill=0.0, base=14, pattern=[[-90, 2], [15, 12]],
                            channel_multiplier=-1)
    kcorr = float(1.0 / (15 * 15 * (T - 1)))
    fcw = W.tile([78, 64], F32, tag="fcw")
    dma.dma_start(out=fcw, in_=D["fc_w"])
    fcwk = W.tile([78, 64], F32, tag="fcwk")
    nc.vector.tensor_scalar_mul(fcwk, fcw, kcorr)
    fcb = W.tile([64, 1], F32, tag="fcb")
    dma.dma_start(out=fcb, in_=D["fc_b"].unsqueeze(1))

    fu1T = W.tile([128, 2, 128], F32, tag="fu1T")
    nc.vector.memset(fu1T[:, 1, :], 0.0)
    dma.dma_start(out=fu1T[:, 0, :], in_=D["fus1_w"][0:128, :])
    dma.dma_start(out=fu1T[0:64, 1, :], in_=D["fus1_w"][128:192, :])
    fu1b = W.tile([128, 1], F32, tag="fu1b")
    dma.dma_start(out=fu1b, in_=D["fus1_b"].unsqueeze(1))
    flg = W.tile([128, 1], F32, tag="flg")
    dma.dma_start(out=flg, in_=D["fus_ln_g"].unsqueeze(1))
    flb = W.tile([128, 1], F32, tag="flb")
    dma.dma_start(out=flb, in_=D["fus_ln_b"].unsqueeze(1))
    fu2T = W.tile([128, 64], F32, tag="fu2T")
    dma.dma_start(out=fu2T, in_=D["fus2_w"])
    fu2b = W.tile([64, 1], F32, tag="fu2b")
    dma.dma_start(out=fu2b, in_=D["fus2_b"].unsqueeze(1))

    epst = W.tile([128, 1], F32, tag="epst")
    nc.vector.memset(epst, EPS)

    bm_all = W.tile([12, S, 12], F32, tag="bm_all")
    fus_t = W.tile([128, S], F32, tag="fus_t")
    fus_f = W.tile([64, S], F32, tag="fus_f")

    # ================= per-sample stages =================
    st = [dict() for _ in range(S)]

    def stage_load(s):
        """DMA x, transpose to feature-major, x stats, proj matmul."""
        d = st[s]
        xtm = sb.tile([TCH, 3, C], F32, tag="xtm")
        dma.dma_start(out=xtm, in_=D["x"][s].rearrange("(c p) f -> p c f", p=TCH))
        xtmb = sb.tile([TCH, 3, C], BF16, tag="xtmb")
        nc.vector.tensor_copy(xtmb, xtm)
        xfm = sb.tile([90, 2, 364], BF16, tag="xfm")
        nc.vector.memset(xfm[:, :, 363:364], 0.0)
        for g in range(2):
            xps = pa([90, 3, 128], BF16)
            for c in range(3):
                nc.tensor.transpose(xps[:, c, 0:TCH],
                                    xtmb[:, c, g * 90:(g + 1) * 90],
                                    identb[0:TCH, 0:TCH])
            nc.vector.tensor_copy(
                xfm[:, g, 0:363].rearrange("a (c t) -> a c t", c=3),
                xps[:, :, 0:TCH])
        xst = tiny.tile([90, 2, 3, 6], F32, tag="xst")
        for g in range(2):
            for c in range(3):
                nc.vector.bn_stats(
                    xst[:, g, c],
                    xfm[:, g, c * TCH:(c + 1) * TCH])
        xmv = tiny.tile([90, 2, 2], F32, tag="xmv")
        for g in range(2):
            nc.vector.bn_aggr(xmv[:, g], xst[:, g])
        d["xfm"], d["xmv"] = xfm, xmv
        hps = pa([96, 512])
        for g in range(2):
            nc.tensor.matmul(hps[:, 0:364], wproj[:, g, :], xfm[:, g],
                             start=(g == 0), stop=(g == 1))
        d["hps"] = hps

    def stage_hpad(s):
        """[G] gelu(proj + bias) into padded conv input row."""
        d = st[s]
        hpad = sb.tile([96, PAD + T + PAD + 5], F32R, tag="hpad")
        nc.vector.memset(hpad.bitcast(F32)[:, 0:PAD], 0.0)
        nc.vector.memset(hpad.bitcast(F32)[:, PAD + T:], 0.0)
        nc.scalar.activation(hpad[:, PAD:PAD + T], d["hps"][:, 0:T], AF.Gelu,
                             bias=bproj, scale=1.0)
        d["hpad"] = hpad
        if DEBUG and s < DBG_SAMPLES:
            dbg(f"h{s}", hpad.bitcast(F32)[:, PAD:PAD + T], (96, T))

    def stage_conv(s):
        """dw-folded-into-pw conv (no bias), cast to bf16, per-channel stats,
        group aggregation. Conv bias folds into the GN affine."""
        d = st[s]
        ysb = sb.tile([96, 3, 364], BF16, tag="ysb")
        for k in range(3):
            dd = DILS[k]
            yps = pa([96, 512])
            for j in range(7):
                off = PAD + (j - 3) * dd
                nc.tensor.matmul(yps[:, 0:364], wconv[k][:, j, :],
                                 d["hpad"][:, off:off + 364],
                                 start=(j == 0), stop=(j == 6))
            nc.vector.tensor_copy(ysb[:, k, 0:T], yps[:, 0:T])
        yst = tiny.tile([96, 3, 6], F32, tag="yst")
        for k in range(3):
            nc.vector.bn_stats(yst[:, k], ysb[:, k, 0:T])
        ymv = tiny.tile([96, 3, 2], F32, tag="ymv")
        for k in range(3):
            nc.vector.bn_aggr(ymv[:, k], yst[:, k])
        # st6: [mean + conv_bias, E[(y+b)^2]] per channel
        st6 = tiny.tile([96, 6], F32, tag="st6")
        nc.vector.tensor_tensor(st6[:, 0:3], ymv[:, :, 0], cb, op=ALU.add)
        nc.vector.tensor_tensor(st6[:, 3:6], st6[:, 0:3], st6[:, 0:3],
                                op=ALU.mult)
        nc.vector.tensor_tensor(st6[:, 3:6], st6[:, 3:6], ymv[:, :, 1],
                                op=ALU.add)
        gst_ps = pa([8, 6])
        nc.tensor.matmul(gst_ps, wgrp, st6, start=True, stop=True)
        gst = tiny.tile([8, 6], F32, tag="gst")
        nc.vector.tensor_copy(gst, gst_ps)
        gvar = tiny.tile([8, 3], F32, tag="gvar")
        nc.vector.tensor_tensor(gvar, gst[:, 0:3], gst[:, 0:3], op=ALU.mult)
        nc.vector.tensor_tensor(gvar, gst[:, 3:6], gvar, op=ALU.subtract)
        d["ysb"], d["gst"], d["gvar"] = ysb, gst, gvar

    def stage_gn_rsqrt(s):
        """[LnExp] group inv-std = exp(-0.5 ln(var+eps)); also corr channel
        inv-std from x stats."""
        d = st[s]
        gln = tiny.tile([8, 3], F32, tag="gln")
        nc.scalar.activation(gln, d["gvar"], AF.Ln, bias=epst[0:8, :], scale=1.0)
        nc.scalar.activation(d["gst"][:, 3:6], gln, AF.Exp, scale=-0.5)
        # corr: cinv = 1/max(sqrt(var*T/(T-1)), 1e-8)
        cvar = tiny.tile([90, 2], F32, tag="cvar")
        nc.vector.tensor_scalar_max(cvar, d["xmv"][:, :, 1], 1e-16)
        cln = tiny.tile([90, 2], F32, tag="cln")
        nc.scalar.activation(cln, cvar, AF.Ln, bias=0.0,
                             scale=float(T) / (T - 1))
        cinv = tiny.tile([90, 2], F32, tag="cinv")
        nc.scalar.activation(cinv, cln, AF.Exp, scale=-0.5)
        d["cinv"] = cinv

    def stage_gn_affine(s):
        """broadcast group stats to channels; per-channel scale/bias with the
        conv bias folded in: out = gelu(y*scl + bia),
        scl = g/sd_g, bia = gnb - (M_g - cb)*scl."""
        d = st[s]
        bc_ps = pa([96, 6])
        nc.tensor.matmul(bc_ps, wbc, d["gst"], start=True, stop=True)
        bc = tiny.tile([96, 6], F32, tag="bc")
        nc.vector.tensor_copy(bc, bc_ps)
        scl = tiny.tile([96, 3], F32, tag="scl")
        nc.vector.tensor_tensor(scl, gng, bc[:, 3:6], op=ALU.mult)
        bia = tiny.tile([96, 3], F32, tag="bia")
        nc.vector.tensor_tensor(bia, bc[:, 0:3], cb, op=ALU.subtract)
        nc.vector.tensor_tensor(bia, bia, scl, op=ALU.mult)
        nc.vector.tensor_tensor(bia, gnb, bia, op=ALU.subtract)
        d["scl"], d["bia"] = scl, bia

    def stage_cat(s):
        """[G] cat = gelu(GN(y)) in bf16."""
        d = st[s]
        cat = sb.tile([96, 3, 364], BF16, tag="cat")
        nc.vector.memset(cat[:, :, 363:364], 0.0)
        for k in range(3):
            nc.scalar.activation(cat[:, k, 0:T], d["ysb"][:, k, 0:T], AF.Gelu,
                                 bias=d["bia"][:, k:k + 1],
                                 scale=d["scl"][:, k:k + 1])
        d["cat"] = cat
        if DEBUG and s < DBG_SAMPLES:
            dbg(f"cat{s}", cat[:, :, 0:T], (96, 3 * T))

    def stage_merge(s):
        """merge matmul, bias, transpose to token-major, LN stats."""
        d = st[s]
        ups = pa([128, 512])
        for g in range(3):
            nc.tensor.matmul(ups[:, 0:364], mw[g], d["cat"][:, g],
                             start=(g == 0), stop=(g == 2))
        ufm = sb.tile([128, 364], BF16, tag="ufm")
        nc.scalar.activation(ufm[:, 0:T], ups[:, 0:T], AF.Identity,
                             bias=mb, scale=1.0)
        tpm = pa([TCH, 3, 128], BF16)
        for c in range(3):
            nc.tensor.transpose(tpm[:, c, :],
                                ufm[:, c * TCH:(c + 1) * TCH], identb)
        mst = tiny.tile([TCH, 3, 6], F32, tag="mst")
        for c in range(3):
            nc.vector.bn_stats(mst[:, c], tpm[:, c])
        mmv = tiny.tile([TCH, 3, 2], F32, tag="mmv")
        for c in range(3):
            nc.vector.bn_aggr(mmv[:, c], mst[:, c])
        d["tpm"], d["mmv"] = tpm, mmv

    def stage_mln_rsqrt(s):
        """[LnExp] token inv-std for merge LN."""
        d = st[s]
        mln_t = tiny.tile([TCH, 3], F32, tag="mln_t")
        nc.scalar.activation(mln_t, d["mmv"][:, :, 1], AF.Ln,
                             bias=epst[0:TCH, :], scale=1.0)
        mrs = tiny.tile([TCH, 3], F32, tag="mrs")
        nc.scalar.activation(mrs, mln_t, AF.Exp, scale=-0.5)
        d["mrs"] = mrs

    def stage_h0(s):
        """normalize, transpose back, [G] h0 = gelu(hat*g+b) in bf16."""
        d = st[s]
        hatm = sb.tile([TCH, 3, 128], BF16, tag="hatm")
        for c in range(3):
            nc.vector.tensor_scalar(hatm[:, c], d["tpm"][:, c],
                                    d["mmv"][:, c, 0:1], d["mrs"][:, c:c + 1],
                                    op0=ALU.subtract, op1=ALU.mult)
        tp2 = pa([128, 3, 128], BF16)
        for c in range(3):
            nc.tensor.transpose(tp2[:, c, 0:TCH], hatm[:, c, :],
                                identb[0:TCH, 0:TCH])
        h0 = sb.tile([128, 364], BF16, tag="h0")
        nc.vector.memset(h0[:, 363:364], 0.0)
        nc.scalar.activation(
            h0[:, 0:363].rearrange("a (c t) -> a c t", c=3), tp2[:, :, 0:TCH],
            AF.Gelu, bias=mlnb, scale=mlng)
        d["h0"] = h0
        if DEBUG and s < DBG_SAMPLES:
            dbg(f"h0_{s}", h0[:, 0:T], (128, T))

    def ln_tm(s, src_fm, pfx, keep_hat=False):
        """transpose fm->tm, stats, [LnExp] inv-std, normalize (f32r hat)."""
        d = st[s]
        tp = pa([TCH, 3, 128], BF16)
        for c in range(3):
            nc.tensor.transpose(tp[:, c, :],
                                src_fm[:, c * TCH:(c + 1) * TCH], identb)
        lst = tiny.tile([TCH, 3, 6], F32, tag=f"{pfx}st")
        for c in range(3):
            nc.vector.bn_stats(lst[:, c], tp[:, c])
        lmv = tiny.tile([TCH, 3, 2], F32, tag=f"{pfx}mv")
        for c in range(3):
            nc.vector.bn_aggr(lmv[:, c], lst[:, c])
        lln = tiny.tile([TCH, 3], F32, tag=f"{pfx}ln")
        nc.scalar.activation(lln, lmv[:, :, 1], AF.Ln,
                             bias=epst[0:TCH, :], scale=1.0)
        lrs = tiny.tile([TCH, 3], F32, tag=f"{pfx}rs")
        nc.scalar.activation(lrs, lln, AF.Exp, scale=-0.5)
        pool_ = p2 if keep_hat else sb
        hat = pool_.tile([TCH, 3, 128], BF16, tag=f"{pfx}hat")
        for c in range(3):
            nc.vector.tensor_scalar(hat[:, c], tp[:, c],
                                    lmv[:, c, 0:1], lrs[:, c:c + 1],
                                    op0=ALU.subtract, op1=ALU.mult)
        tpb = pa([128, 3, 128], BF16)
        for c in range(3):
            nc.tensor.transpose(tpb[:, c, 0:TCH], hat[:, c, :],
                                identb[0:TCH, 0:TCH])
        return hat, tpb

    def stage_p2a(s):
        """[LnExp] qkv -> v prep -> scores + exp."""
        d = st[s]
        h0 = d["h0"]
        # ---- qkv ----
        qfm = p2.tile([64, 2, 364], BF16, tag="qfm")
        kfm = p2.tile([64, 2, 364], BF16, tag="kfm")
        vfm = p2.tile([128, 364], BF16, tag="vfm")
        nc.vector.memset(qfm[:, :, 363:364], 0.0)
        nc.vector.memset(kfm[:, :, 363:364], 0.0)
        qps = pa([128, 512])
        nc.tensor.matmul(qps[:, 0:364], qkvT[0], h0, start=True, stop=True)
        for i in range(2):
            nc.scalar.activation(qfm[:, i, 0:T],
                                 qps[i * 64:(i + 1) * 64, 0:T],
                                 AF.Identity, bias=qb_s[i * 64:(i + 1) * 64, :],
                                 scale=ISQ)
        kps = pa([128, 512])
        nc.tensor.matmul(kps[:, 0:364], qkvT[1], h0, start=True, stop=True)
        for i in range(2):
            nc.vector.tensor_scalar(kfm[:, i, 0:T],
                                    kps[i * 64:(i + 1) * 64, 0:T],
                                    1.0, qb3[i * 64:(i + 1) * 64, 1:2],
                                    op0=ALU.mult, op1=ALU.add)
        vps = pa([128, 512])
        nc.tensor.matmul(vps[:, 0:364], qkvT[2], h0, start=True, stop=True)
        nc.vector.tensor_scalar(vfm[:, 0:T], vps[:, 0:T], 1.0, qb3[:, 2:3],
                                op0=ALU.mult, op1=ALU.add)
        vtp = pa([TCH, 3, 128], BF16)
        for c in range(3):
            nc.tensor.transpose(vtp[:, c, :], vfm[:, c * TCH:(c + 1) * TCH],
                                identb)
        vtm = p2.tile([TCH, 3, 4, 33], BF16, tag="vtm")
        nc.vector.tensor_copy(vtm[:, :, :, 0:32],
                              vtp.rearrange("p c (h d) -> p c h d", h=4))
        nc.vector.memset(vtm[:, :, :, 32:33], 1.0)

        # ---- scores (transposed) + exp (bf16) ----
        expt = p2.tile([TCH, 3, 2, 2, 364], BF16, tag="expt")
        for cs in range(3):
            for b_ in range(2):
                scps = pa([TCH, 2, 512])
                for i in range(2):
                    nc.tensor.matmul(
                        scps[:, i, 0:364],
                        kfm[b_ * 32:(b_ + 1) * 32, i,
                            cs * TCH:(cs + 1) * TCH],
                        qfm[b_ * 32:(b_ + 1) * 32, i, :],
                        start=True, stop=True)
                nc.scalar.activation(
                    expt[:, cs, b_], scps[:, :, 0:364], AF.Exp)

        d["vtm"], d["expt"] = vtm, expt

    def stage_p2b(s):
        """AV -> softmax Z -> normalize -> out-proj -> residual."""
        d = st[s]
        h0 = d["h0"]
        vtm, expt = d["vtm"], d["expt"]
        # ---- AV: V stationary (33-col LDW), exp moving N=364; out is
        # feature-major per-head [33,364] with Z on row 32; heads packed two
        # per 1-bank psum tile at partition offsets 0/64. ----
        avps = [pa([97, 512]), pa([97, 512])]
        for h in range(4):
            tile_ = avps[h // 2]
            po = (h % 2) * 64
            for cs in range(3):
                nc.tensor.matmul(
                    tile_[po:po + 33, 0:364],
                    vtm[:, cs, h, :],
                    expt[:, cs, h % 2, h // 2, :],
                    start=(cs == 0), stop=(cs == 2))
        # 1/Z rows -> one sbuf row-tile (free-dim stacked), PE-broadcast to
        # 32 partitions per head, then normalize O per head.
        zln = p2.tile([1, 4, 364], F32, tag="zln")
        for h in range(4):
            nc.scalar.activation(
                zln[:, h, :],
                avps[h // 2][(h % 2) * 64 + 32:(h % 2) * 64 + 33, 0:364],
                AF.Ln)
        rzrow = p2.tile([1, 4, 364], BF16, tag="rzrow")
        nc.scalar.activation(rzrow, zln, AF.Exp, scale=-1.0)
        rzb = pa([97, 2, 512])
        for h in range(4):
            nc.tensor.matmul(rzb[(h % 2) * 64:(h % 2) * 64 + 32, h // 2, 0:364],
                             ones32, rzrow[:, h, :], start=True, stop=True)
        rzbs = p2.tile([97, 2, 364], BF16, tag="rzbs")
        nc.vector.tensor_copy(rzbs[0:32], rzb[0:32, :, 0:364])
        nc.vector.tensor_copy(rzbs[64:96], rzb[64:96, :, 0:364])
        oat = p2.tile([97, 2, 364], BF16, tag="oat")
        nc.vector.memset(oat, 0.0)
        for h in range(4):
            po = (h % 2) * 64
            nc.vector.tensor_tensor(oat[po:po + 32, h // 2, :],
                                    avps[h // 2][po:po + 32, 0:364],
                                    rzbs[po:po + 32, h // 2, :], op=ALU.mult)

        # ---- out proj (4 per-head accumulating matmuls) + residual ----
        rps = pa([128, 512])
        for p_ in range(2):
            nc.tensor.matmul(rps[:, 0:364], owT_s[p_], oat[:, p_, :],
                             start=(p_ == 0), stop=(p_ == 1))
        rfm = p2.tile([128, 364], BF16, tag="rfm")
        nc.vector.scalar_tensor_tensor(rfm[:, 0:T], rps[:, 0:T], ob,
                                       h0[:, 0:T], op0=ALU.add, op1=ALU.add)
        if DEBUG and s < DBG_SAMPLES:
            dbg(f"rfm{s}", rfm[:, 0:T], (128, T))
        d["rfm"] = rfm

    def stage_p2c(s):
        """ln1 -> ffn -> residual."""
        d = st[s]
        rfm = d["rfm"]
        hat1, tpb1 = ln_tm(s, rfm, "l1", keep_hat=False)
        h1 = p2.tile([128, 364], BF16, tag="h1")
        nc.vector.memset(h1[:, 363:364], 0.0)
        nc.scalar.activation(
            h1[:, 0:363].rearrange("a (c t) -> a c t", c=3), tpb1[:, :, 0:TCH],
            AF.Identity, bias=ln1b, scale=ln1g)

        # ---- ffn ----
        g1 = p2.tile([128, 2, 364], BF16, tag="g1")
        nc.vector.memset(g1[:, :, 363:364], 0.0)
        for i in range(2):
            f1ps = pa([128, 512])
            nc.tensor.matmul(f1ps[:, 0:364], f1T[i], h1,
                             start=True, stop=True)
            nc.scalar.activation(g1[:, i, 0:T], f1ps[:, 0:T], AF.Relu,
                                 bias=f1b[:, i:i + 1], scale=1.0)
        f2ps = pa([128, 512])
        for i in range(2):
            nc.tensor.matmul(f2ps[:, 0:364], f2T[i], g1[:, i],
                             start=(i == 0), stop=(i == 1))
        ffo = p2.tile([128, 364], BF16, tag="ffo")
        nc.vector.scalar_tensor_tensor(ffo[:, 0:T], f2ps[:, 0:T], f2b,
                                       h1[:, 0:T], op0=ALU.add, op1=ALU.add)
        d["ffo"] = ffo

    def stage_p2d(s):
        """ln2 -> pooling -> correlation gram."""
        d = st[s]
        ffo = d["ffo"]
        hat2, tpb2 = ln_tm(s, ffo, "l2", keep_hat=True)
        h2 = p2.tile([128, 364], BF16, tag="h2")
        nc.vector.memset(h2[:, 363:364], 0.0)
        nc.scalar.activation(
            h2[:, 0:363].rearrange("a (c t) -> a c t", c=3), tpb2[:, :, 0:TCH],
            AF.Identity, bias=ln2b, scale=ln2g)
        if DEBUG and s < DBG_SAMPLES:
            dbg(f"h2_{s}", h2[:, 0:T], (128, T))

        # ---- attentive pooling ----
        plps = pa([1, 512])
        nc.tensor.matmul(plps[:, 0:364], poolw, h2, start=True, stop=True)
        pw_sb = p2.tile([1, T], F32, tag="pw_sb")
        zp = tiny.tile([1, 1], F32, tag="zp")
        nc.scalar.activation(pw_sb, plps[:, 0:T], AF.Exp,
                             bias=poolb, scale=1.0, accum_out=zp)
        rzp = tiny.tile([1, 1], F32, tag="rzp")
        nc.vector.reciprocal(rzp, zp)
        wn = p2.tile([1, T], BF16, tag="wn")
        nc.vector.tensor_scalar_mul(wn, pw_sb, rzp)
        wtp = pa([TCH, 3, 2], BF16)
        for c in range(3):
            nc.tensor.transpose(wtp[:, c, 0:1],
                                wn[:, c * TCH:(c + 1) * TCH],
                                identb[0:1, 0:1])
        wcol = tiny.tile([TCH, 3, 1], BF16, tag="wcol")
        nc.vector.tensor_copy(wcol, wtp[:, :, 0:1])
        tps = pa([128, 1])
        for c in range(3):
            nc.tensor.matmul(tps, hat2[:, c, :], wcol[:, c, :],
                             start=(c == 0), stop=(c == 2))
        nc.vector.tensor_scalar(fus_t[:, s:s + 1], tps, ln2g, ln2b,
                                op0=ALU.mult, op1=ALU.add)

        # ---- correlation fingerprint ----
        wcorr = p2.tile([90, 2, 12], BF16, tag="wcorr")
        for g in range(2):
            nc.vector.tensor_scalar_mul(wcorr[:, g], cmask[:, g],
                                        d["cinv"][:, g:g + 1])
        swps = pa([12, 512])
        for g in range(2):
            nc.tensor.matmul(swps[:, 0:364], wcorr[:, g], d["xfm"][:, g],
                             start=(g == 0), stop=(g == 1))
        swsb = p2.tile([12, T], BF16, tag="swsb")
        rsum = tiny.tile([12, 1], F32, tag="rsum")
        nc.vector.tensor_scalar(swsb, swps[:, 0:T], 1.0, 0.0, op0=ALU.mult,
                                op1=ALU.add, accum_out=rsum)
        swtp = pa([TCH, 3, 12], BF16)
        for c in range(3):
            nc.tensor.transpose(swtp[:, c, :], swsb[:, c * TCH:(c + 1) * TCH],
                                identb[0:12, 0:12])
        swtm = tiny.tile([TCH, 3, 12], BF16, tag="swtm")
        nc.vector.tensor_copy(swtm, swtp)
        rsT_ps = pa([1, 12])
        nc.tensor.transpose(rsT_ps, rsum, ident[0:12, 0:12])
        rsT = tiny.tile([1, 12], F32, tag="rsT")
        nc.vector.tensor_copy(rsT, rsT_ps)
        rsTn = tiny.tile([1, 12], F32, tag="rsTn")
        nc.vector.tensor_scalar_mul(rsTn, rsT, -1.0 / T)
        gps = pa([12, 12])
        for c in range(3):
            nc.tensor.matmul(gps, swtm[:, c, :], swtm[:, c, :],
                             start=(c == 0), stop=False)
        nc.tensor.matmul(gps, rsTn, rsT, start=False, stop=True)
        nc.vector.tensor_copy(bm_all[:, s, :], gps)

    # ================= block-scheduled emission =================
    for b0 in range(0, S, BLK):
        blk = range(b0, min(b0 + BLK, S))
        for s in blk:
            stage_load(s)
        for s in blk:
            stage_hpad(s)       # [G]
        for s in blk:
            stage_conv(s)
        for s in blk:
            stage_gn_rsqrt(s)   # [LnExp]
        for s in blk:
            stage_gn_affine(s)
        for s in blk:
            stage_cat(s)        # [G]
        for s in blk:
            stage_merge(s)
        for s in blk:
            stage_mln_rsqrt(s)  # [LnExp]
        for s in blk:
            stage_h0(s)         # [G]
        bl = list(blk)
        for pair in [bl[i:i + 2] for i in range(0, len(bl), 2)]:
            for s in pair:
                stage_p2a(s)    # [LnExp]
            for s in pair:
                stage_p2b(s)
            for s in pair:
                stage_p2c(s)
            for s in pair:
                stage_p2d(s)

    # ================= batched tail =================
    bm_dram = nc.dram_tensor("bm_scratch", [12, S, 12], F32).ap()
    dma.dma_start(out=bm_dram, in_=bm_all)
    fcv = W.tile([78, S], F32, tag="fcv")
    row_off = 0
    for i in range(12):
        n = 12 - i
        dma.dma_start(
            out=fcv[row_off:row_off + n, :],
            in_=bm_dram[i, :, i:12].transpose([1, 0]))
        row_off += n
    fcps = pa([64, S])
    nc.tensor.matmul(fcps, fcwk, fcv, start=True, stop=True)
    nc.scalar.activation(fus_f, fcps, AF.Gelu, bias=fcb, scale=1.0)

    fu_ps = pa([128, S])
    nc.tensor.matmul(fu_ps, fu1T[:, 0, :], fus_t, start=True, stop=False)
    nc.tensor.matmul(fu_ps, fu1T[0:64, 1, :], fus_f, start=False, stop=True)
    zfm = W.tile([128, S], F32, tag="zfm")
    nc.vector.tensor_scalar(zfm, fu_ps, 1.0, fu1b, op0=ALU.mult, op1=ALU.add)
    ztp = pa([S, 128])
    nc.tensor.transpose(ztp, zfm, ident)
    ztm = W.tile([S, 128], F32, tag="ztm")
    nc.vector.tensor_copy(ztm, ztp)
    zst = W.tile([S, 6], F32, tag="zst")
    nc.vector.bn_stats(zst, ztm)
    zmv = W.tile([S, 2], F32, tag="zmv")
    nc.vector.bn_aggr(zmv, zst)
    zln = W.tile([S, 1], F32, tag="zln")
    nc.scalar.activation(zln, zmv[:, 1:2], AF.Ln, bias=epst[0:S, :], scale=1.0)
    zrs = W.tile([S, 1], F32, tag="zrs")
    nc.scalar.activation(zrs, zln, AF.Exp, scale=-0.5)
    zhat = W.tile([S, 128], F32, tag="zhat")
    nc.vector.tensor_scalar(zhat, ztm, zmv[:, 0:1], zrs,
                            op0=ALU.subtract, op1=ALU.mult)
    zhtp = pa([128, S])
    nc.tensor.transpose(zhtp, zhat, ident[0:S, 0:S])
    zg = W.tile([128, S], F32, tag="zg")
    nc.scalar.activation(zg, zhtp, AF.Gelu, bias=flb, scale=flg)
    out_ps = pa([64, S])
    nc.tensor.matmul(out_ps, fu2T, zg, start=True, stop=True)
    out_sb = W.tile([64, S], F32, tag="out_sb")
    nc.scalar.activation(out_sb, out_ps, AF.Identity, bias=fu2b, scale=1.0)
    outT_ps = pa([S, 64])
    nc.tensor.transpose(outT_ps, out_sb, ident[0:64, 0:64])
    outT = W.tile([S, 64], F32, tag="outT")
    nc.vector.tensor_copy(outT, outT_ps)
    dma.dma_start(out=out_dram, in_=outT)

    for p in reversed(pools):
        p.__exit__(None, None, None)


_PROGRAM = None


def _get_program():
    global _PROGRAM
    if _PROGRAM is None:
        _PROGRAM = build_program()
    return _PROGRAM


def kernel(**inputs):
    from concourse.bass_utils import run_bass_kernel_spmd

    nc, _ = _get_program()
    in_maps = []
    for c in range(NCORES):
        m = {}
        for name, _shape in INPUT_SPECS:
            if name == "x":
                m["x"] = np.ascontiguousarray(
                    np.asarray(inputs["x"][c * S:(c + 1) * S], dtype=np.float32))
            else:
                m[name] = np.ascontiguousarray(
                    np.asarray(inputs[name], dtype=np.float32))
        in_maps.append(m)
    res = run_bass_kernel_spmd(nc, in_maps, list(range(NCORES)))
    global LAST_RESULTS
    LAST_RESULTS = res
    out = np.concatenate([res.results[c]["out"] for c in range(NCORES)], axis=0)
    return out.astype(np.float32)


LAST_RESULTS = None
